# revision 1
# baseline (speedup 1.0000x reference)
"""Trainium2 Bass kernel for nn_BoundaryModule_38422777430159.

Reference computation (B=4, C=256, T=256, N=10, D=40, DIM0=512, DIM1=128):
  x1 = sample(feature)            # (B,C,N,D,T) via (T, N*D*T) smp matmul
  x2 = leaky(einsum('bcndt,ocn->bodt', x1, w0) + b0)
  x3 = leaky(w1 @ x2 + b1)        # 1x1 conv
  x4 = leaky(conv3x3(x3, w2) + b2)
  out = sigmoid(w3 @ x4 + b3)     # (B, D, T)

Device strategy (8 NeuronCores, SPMD; core i handles b = i//2 and
t-half th = i%2 with a 1-column halo):
  A[n]   = feature[b].T-contraction with w0[:, :, n]   (PE, fp32r)
  x2     = sum over (n, tau-chunk) of A-tiles @ W-slice (PE, fp32r)
           where W-slice is the dense (2560, 40*130) sampling matrix
           columns for this core's t-window, streamed from HBM
  x3, conv3x3, final 1x1 + sigmoid on-core; output (40, 128) per core.

fp32r (TF32-like fast fp32 path, 1 cyc/row vs 4 for fp32) is used for all
matmuls; PSUM accumulates in fp32.
"""
import os
import sys

for _p in ("/opt/trn_rl_repo", "/root/.axon_site/_ro/trn_rl_repo"):
    if os.path.isdir(_p) and _p not in sys.path:
        sys.path.append(_p)

import numpy as np

import concourse.bass as bass
import concourse.tile as tile
from concourse import mybir
from concourse.bass_utils import run_bass_kernel_spmd
from concourse.tile_rust import add_dep_helper

T = 256
N = 10
D = 40
B = 4
C_IN = 256
DIM0 = 512
DIM1 = 128

TW = 130          # t-window incl. 1-col halo each side
COLS = D * TW     # 5200 matmul columns per core
FW = 400          # free-dim chunk (<=512 psum bank, >=256 keeps fp32r fast)
NF = COLS // FW   # 13
K = 2 * N         # 20 contraction chunks of 128 (tau-chunk major within n)
DCH = 3           # conv d-rows per psum group
NDCH = (D + DCH - 1) // DCH  # 14 (13*3 + 1)

F32 = mybir.dt.float32
F32R = mybir.dt.float32r


def _legalize_waits(nc, limit=1):
    """This walrus build allows a single embedded sync wait per real
    instruction; move the excess onto standalone NoOp wait-carriers."""
    moved = 0
    for f in nc.m.functions:
        for bb in f.blocks:
            il = bb.instructions
            out = []
            changed = False
            for inst in il:
                si = inst.sync_info
                ty = type(inst).__name__
                if (si and si.on_wait and len(si.on_wait) > limit
                        and ty not in ("InstEventSemaphore", "InstNoOp")):
                    keep = si.on_wait[-limit:]
                    for w in si.on_wait[:-limit]:
                        out.append(mybir.InstNoOp(
                            name=f"waitnop-{nc.next_id()}",
                            sync_info=mybir.SyncInfo(on_wait=[w], on_update=[]),
                            bass_nofuse=True,
                            engine=inst.engine,
                        ))
                        moved += 1
                    inst.sync_info = mybir.SyncInfo(
                        on_wait=keep, on_update=si.on_update)
                    changed = True
                out.append(inst)
            if changed:
                bb.instructions = out
    return moved


def _build_program(keep=None, debug=False):
    if keep is None:
        keep = tuple(tuple(range(K)) for _ in range(NF))
    nc = bass.Bass(trn_type="TRN2")
    MAX = mybir.AluOpType.max
    MULT = mybir.AluOpType.mult

    feat_d = nc.dram_tensor("feat", [C_IN, T], F32R, kind="ExternalInput")
    w0_d = nc.dram_tensor("w0t", [N, C_IN, DIM0], F32R, kind="ExternalInput")
    wsmp_d = nc.dram_tensor("wsmp", [NF, K, 128, FW], F32R,
                            kind="ExternalInput")
    w1_d = nc.dram_tensor("w1t", [DIM0, DIM1], F32R, kind="ExternalInput")
    w2_d = nc.dram_tensor("w2t", [9, DIM1, DIM1], F32R, kind="ExternalInput")
    w3_d = nc.dram_tensor("w3t", [DIM1, 1], F32R, kind="ExternalInput")
    b0_d = nc.dram_tensor("b0", [4, 128, 1], F32, kind="ExternalInput")
    b1_d = nc.dram_tensor("b1", [128, 1], F32, kind="ExternalInput")
    b2_d = nc.dram_tensor("b2", [128, 1], F32, kind="ExternalInput")
    b3_d = nc.dram_tensor("b3", [1, 1], F32, kind="ExternalInput")
    out_d = nc.dram_tensor("out", [1, D * TW], F32, kind="ExternalOutput")
    if debug:
        dbg_a = nc.dram_tensor("dbg_a", [K, 128, DIM0], F32, kind="ExternalOutput")
        dbg_x2 = nc.dram_tensor("dbg_x2", [4, 128, FW], F32, kind="ExternalOutput")
        dbg_x3 = nc.dram_tensor("dbg_x3", [128, COLS], F32, kind="ExternalOutput")
        dbg_x4 = nc.dram_tensor("dbg_x4", [128, DCH * TW], F32, kind="ExternalOutput")

    with tile.TileContext(nc) as tc:
        with (
            tc.tile_pool(name="inp", bufs=1) as inp,
            tc.tile_pool(name="wst", bufs=24) as wst,
            tc.tile_pool(name="apool", bufs=1) as apool,
            tc.tile_pool(name="x2p", bufs=2) as x2p,
            tc.tile_pool(name="x3p", bufs=1) as x3p,
            tc.tile_pool(name="x4p", bufs=2) as x4p,
            tc.tile_pool(name="scr", bufs=2) as scr,
            tc.tile_pool(name="outp", bufs=1) as outp,
            tc.tile_pool(name="psb", bufs=1, space="PSUM") as psb,
            tc.tile_pool(name="psg", bufs=2, space="PSUM") as psg,
        ):
            # ---- input DMAs (all destinations write-once) ----
            feat = [inp.tile([128, T], F32R, tag=f"feat{c}", name=f"feat{c}")
                    for c in range(2)]
            for c in range(2):
                nc.sync.dma_start(feat[c][:], feat_d[c * 128:(c + 1) * 128, :])
            w0t = []
            w0_dmas = []
            for n in range(N):
                pair = []
                for c in range(2):
                    t_ = inp.tile([128, DIM0], F32R, tag=f"w0_{n}_{c}",
                                  name=f"w0_{n}_{c}")
                    w0_dmas.append(nc.sync.dma_start(
                        t_[:], w0_d[n, c * 128:(c + 1) * 128, :]))
                    pair.append(t_)
                w0t.append(pair)
            # prefetch the first two f-chunks of the W stream, each tile
            # ordered 1:1 behind the matching w0 load so stage A and stage B
            # both trickle-start as DMAs land
            wpre = {}
            for k in keep[0]:
                wt = wst.tile([128, FW], F32R, tag="w", name=f"wt_0_{k}")
                dma = nc.sync.dma_start(wt[:], wsmp_d[0, k])
                add_dep_helper(dma.ins, w0_dmas[k].ins,
                               reason="interleave W stream with w0")
                wpre[(0, k)] = wt
            w1t = []
            for c in range(4):
                t_ = inp.tile([128, DIM1], F32R, tag=f"w1_{c}", name=f"w1_{c}")
                nc.sync.dma_start(t_[:], w1_d[c * 128:(c + 1) * 128, :])
                w1t.append(t_)
            w2t = []
            for j in range(9):
                t_ = inp.tile([128, DIM1], F32R, tag=f"w2_{j}", name=f"w2_{j}")
                nc.sync.dma_start(t_[:], w2_d[j])
                w2t.append(t_)
            w3t = inp.tile([128, 1], F32R, tag="w3", name="w3t_sb")
            nc.sync.dma_start(w3t[:], w3_d[:])
            b0t = inp.tile([128, 4], F32, tag="b0", name="b0_sb")
            nc.sync.dma_start(b0t[:].rearrange("p (a b) -> p a b", b=1),
                              b0_d[:].transpose((1, 0, 2)))
            b1t = inp.tile([128, 1], F32, tag="b1", name="b1_sb")
            nc.sync.dma_start(b1t[:], b1_d[:])
            b2t = inp.tile([128, 1], F32, tag="b2", name="b2_sb")
            nc.sync.dma_start(b2t[:], b2_d[:])
            b3t = inp.tile([1, 1], F32, tag="b3", name="b3_sb")
            nc.sync.dma_start(b3t[:], b3_d[:])

            # ---- teach engines the input-DMA ticks (1 wait per inst) ----
            dve_scr = scr.tile([128, 4], F32, tag="dscr", name="dve_scr")
            nc.vector.tensor_copy(dve_scr[:, 0:1], b1t[:])
            nc.vector.tensor_copy(dve_scr[:, 1:2], b2t[:])
            nc.vector.tensor_copy(dve_scr[:, 2:3], b0t[:, 0:1])
            nc.scalar.mul(dve_scr[0:1, 3:4], b3t[:], 1.0)
            # one warm-up accumulation group, spread so stage A can start as
            # soon as the tiles it needs have landed
            warm = psg.tile([1, 4], F32, tag="g", name="warm_ps")

            def warm_mm(t_, first, last):
                nc.tensor.matmul(warm[:], t_[:, 0:1], t_[:, 0:4],
                                 start=first, stop=last)

            for i, t_ in enumerate(feat):
                warm_mm(t_, i == 0, False)

            # ---- stage A: A[k] = (feature chunk).T @ w0_n  -> [tau, o] ----
            atiles = []
            for n in range(N):
                warm_mm(w0t[n][0], False, False)
                warm_mm(w0t[n][1], False, False)
                for tch in range(2):
                    ps = psb.tile([128, DIM0], F32, tag=f"b{tch}",
                                  name=f"psa{n}_{tch}")
                    for c in range(2):
                        nc.tensor.matmul(
                            ps[:],
                            feat[c][:, tch * 128:(tch + 1) * 128],
                            w0t[n][c][:],
                            start=(c == 0), stop=(c == 1),
                        )
                    at = apool.tile([128, DIM0], F32R, tag=f"a{n}_{tch}",
                                    name=f"a{n}_{tch}")
                    nc.vector.tensor_copy(at[:], ps[:])
                    atiles.append(at)
                    if debug:
                        nc.sync.dma_start(dbg_a[n * 2 + tch],
                                          at[:].bitcast(F32))

            # ---- stages B (sampling contraction) + C (1x1) per f-chunk ----
            # och pairs double-buffered in PSUM so consecutive f-chunks overlap
            x3 = x3p.tile([128, COLS], F32R, tag="x3", name="x3_sb")
            for f in range(NF):
                ks = list(keep[f])
                wts = {}
                x2c = [None] * 4
                for g in range(2):
                    a0 = psb.tile([128, FW], F32, tag=f"b{2 * g}",
                                  name=f"psb{f}_{2 * g}")
                    a1 = psb.tile([128, FW], F32, tag=f"b{2 * g + 1}",
                                  name=f"psb{f}_{2 * g + 1}")
                    for k in ks:
                        if g == 0:
                            if f < 1:
                                wt = wpre[(f, k)]
                            else:
                                wt = wst.tile([128, FW], F32R, tag="w",
                                              name=f"wt_{f}_{k}")
                                nc.sync.dma_start(wt[:], wsmp_d[f, k])
                            wts[k] = wt
                        wt = wts[k]
                        for o, acc in ((2 * g, a0), (2 * g + 1, a1)):
                            nc.tensor.matmul(
                                acc[:],
                                atiles[k][:, o * 128:(o + 1) * 128],
                                wt[:],
                                start=(k == ks[0]), stop=(k == ks[-1]),
                            )
                    for o, acc in ((2 * g, a0), (2 * g + 1, a1)):
                        yt = x2p.tile([128, FW], F32R, tag=f"x2_{o}",
                                      name=f"x2_{f}_{o}")
                        nc.vector.tensor_scalar_add(yt[:], acc[:],
                                                    b0t[:, o:o + 1])
                        nc.vector.scalar_tensor_tensor(yt[:], yt[:], 0.01,
                                                       yt[:], MULT, MAX)
                        x2c[o] = yt
                        if debug and f == 0:
                            nc.sync.dma_start(dbg_x2[o], yt[:].bitcast(F32))
                if f == 0:
                    # late warm-ups: small weights have landed by now
                    for t_ in w1t:
                        warm_mm(t_, False, False)
                    for j, t_ in enumerate(w2t):
                        warm_mm(t_, False, j == 8)
                psc = psg.tile([128, FW], F32, tag="g", name=f"psc{f}")
                for o in range(4):
                    nc.tensor.matmul(psc[:], w1t[o][:], x2c[o][:],
                                     start=(o == 0), stop=(o == 3))
                x3f = x3[:, f * FW:(f + 1) * FW]
                nc.vector.tensor_scalar_add(x3f, psc[:], b1t[:])
                nc.vector.scalar_tensor_tensor(x3f, x3f, 0.01, x3f, MULT, MAX)

            # ---- stage D: 3x3 conv over (d, t') with zero padding ----
            if debug:
                nc.sync.dma_start(dbg_x3[:], x3[:].bitcast(F32))
            pad = x3p.tile([128, D + 2, TW + 2], F32R, tag="pad", name="padbuf")
            nc.vector.memset(pad[:].bitcast(F32), 0.0)
            x3g = x3[:].rearrange("p (d t) -> p d t", d=D)
            for dc in range(NDCH):
                d0 = dc * DCH
                nd = min(DCH, D - d0)
                nc.vector.tensor_copy(
                    pad[:, 1 + d0:1 + d0 + nd, 1:TW + 1], x3g[:, d0:d0 + nd, :])
            out_sb = outp.tile([1, D * TW], F32, tag="os", name="out_sb")
            x4cs = [None] * NDCH

            def stage_e(dc):
                d0 = dc * DCH
                fw = min(DCH, D - d0) * TW
                pse = psg.tile([1, DCH * TW], F32, tag="g", name=f"pse{dc}")
                nc.tensor.matmul(pse[:, 0:fw], w3t[:], x4cs[dc][:, 0:fw],
                                 start=True, stop=True)
                nc.scalar.activation(
                    out_sb[:, d0 * TW:d0 * TW + fw], pse[:, 0:fw],
                    mybir.ActivationFunctionType.Sigmoid,
                    bias=b3t[:], scale=1.0,
                )

            for dc in range(NDCH):
                d0 = dc * DCH
                nd = min(DCH, D - d0)
                fw = nd * TW
                psd = psg.tile([128, DCH * TW], F32, tag="d", name=f"psd{dc}")
                for j in range(9):
                    dy, dx = j // 3, j % 3
                    nc.tensor.matmul(
                        psd[:, 0:fw],
                        w2t[j][:],
                        pad[:, d0 + dy:d0 + dy + nd, dx:dx + TW],
                        start=(j == 0), stop=(j == 8),
                    )
                x4c = x4p.tile([128, DCH * TW], F32R, tag="x4", name=f"x4_{dc}")
                nc.vector.tensor_scalar_add(x4c[:, 0:fw], psd[:, 0:fw], b2t[:])
                nc.vector.scalar_tensor_tensor(x4c[:, 0:fw], x4c[:, 0:fw],
                                               0.01, x4c[:, 0:fw], MULT, MAX)
                x4cs[dc] = x4c
                if debug and dc == 0:
                    nc.sync.dma_start(dbg_x4[:], x4c[:].bitcast(F32))
                # software pipeline: E for the previous chunk runs after the
                # next conv group is queued, hiding the DVE eviction latency
                if dc >= 1:
                    stage_e(dc - 1)
            stage_e(NDCH - 1)
            nc.scalar.dma_start(out_d[:], out_sb[:])
    _legalize_waits(nc)
    return nc


_PROGRAM = None


def _get_program(keep):
    global _PROGRAM
    if _PROGRAM is None or _PROGRAM[0] != keep:
        _PROGRAM = (keep, _build_program(keep=keep))
    return _PROGRAM[1]


def _prep_inputs(feature, smp_weight, w0, b0, w1, b1, w2, b2, w3, b3):
    feature = np.ascontiguousarray(np.asarray(feature, dtype=np.float32))
    smp = np.asarray(smp_weight, dtype=np.float32).reshape(T, N, D, T)
    w0p = np.ascontiguousarray(
        np.asarray(w0, dtype=np.float32).transpose(2, 1, 0))     # (N, C, DIM0)
    w1p = np.ascontiguousarray(np.asarray(w1, dtype=np.float32).T)  # (512,128)
    w2p = np.ascontiguousarray(
        np.asarray(w2, dtype=np.float32).transpose(2, 3, 1, 0).reshape(
            9, DIM1, DIM1))                                       # (9, C, O)
    w3p = np.ascontiguousarray(np.asarray(w3, dtype=np.float32).T)  # (128,1)
    b0p = np.ascontiguousarray(
        np.asarray(b0, dtype=np.float32).reshape(4, 128, 1))
    b1p = np.asarray(b1, dtype=np.float32).reshape(128, 1)
    b2p = np.asarray(b2, dtype=np.float32).reshape(128, 1)
    b3p = np.asarray(b3, dtype=np.float32).reshape(1, 1)

    # W slices per t-half: columns t' in [t0-1, t0+129), zero-padded outside
    # [0, T). Row-major layout (n, tau) -> K=20 chunks of 128.
    wslices = []
    for th in range(2):
        t0 = th * 128
        lo, hi = t0 - 1, t0 + TW - 1
        clo, chi = max(lo, 0), min(hi, T)
        sl = np.zeros((T, N, D, TW), dtype=np.float32)
        sl[:, :, :, clo - lo:clo - lo + (chi - clo)] = smp[:, :, :, clo:chi]
        sl = sl.transpose(1, 0, 2, 3).reshape(K, 128, D * TW)
        # (NF, K, 128, FW): each streamed [128, FW] tile contiguous in DRAM
        sl = sl.reshape(K, 128, NF, FW).transpose(2, 0, 1, 3)
        wslices.append(np.ascontiguousarray(sl))
    # skip all-zero W tiles; the keep pattern is the union over both t-halves
    # so the single SPMD program stays valid for every core
    nz = (np.abs(wslices[0]).max(axis=(2, 3)) > 0) | \
         (np.abs(wslices[1]).max(axis=(2, 3)) > 0)   # (NF, K)
    keep = tuple(
        tuple(np.nonzero(nz[f])[0].tolist()) or (0,) for f in range(NF))
    return feature, w0p, w1p, w2p, w3p, b0p, b1p, b2p, b3p, wslices, keep


def kernel(feature, smp_weight, w0, b0, w1, b1, w2, b2, w3, b3,
           _trace=False):
    (feature, w0p, w1p, w2p, w3p, b0p, b1p, b2p, b3p, wslices,
     keep) = _prep_inputs(
        feature, smp_weight, w0, b0, w1, b1, w2, b2, w3, b3)

    nc = _get_program(keep)
    in_maps = []
    for core in range(8):
        b, th = core // 2, core % 2
        in_maps.append({
            "feat": np.ascontiguousarray(feature[b]),
            "w0t": w0p,
            "wsmp": wslices[th],
            "w1t": w1p,
            "w2t": w2p,
            "w3t": w3p,
            "b0": b0p,
            "b1": b1p,
            "b2": b2p,
            "b3": b3p,
        })
    res = run_bass_kernel_spmd(nc, in_maps, core_ids=list(range(8)),
                               trace=_trace)
    out = np.empty((B, D, T), dtype=np.float32)
    for core in range(8):
        b, th = core // 2, core % 2
        full = res.results[core]["out"].reshape(D, TW)
        out[b, :, th * 128:(th + 1) * 128] = full[:, 1:TW - 1]
    if _trace:
        return out, res
    return out



# revision 2
# speedup vs baseline: 1.0503x; 1.0503x over previous
"""Trainium2 Bass kernel for nn_BoundaryModule_38422777430159 (v2).

Reference (B=4, C=256, T=256, N=10, D=40, DIM0=512, DIM1=128):
  x1 = sample(feature)            # (B,C,N,D,T) via (T, N*D*T) smp matmul
  x2 = leaky(einsum('bcndt,ocn->bodt', x1, w0) + b0)
  x3 = leaky(w1 @ x2 + b1)
  x4 = leaky(conv3x3(x3, w2) + b2)
  out = sigmoid(w3 @ x4 + b3)     # (B, D, T)

v2 strategy (8 cores SPMD, core = (b, t-half), TW=130 incl halo):
  The sampling matrix columns (k,d,t) have tau-support that is an interval
  moving AFFINELY in t with integer stride shift_k = 27-4k per Tc=18 t-chunk.
  Per core:
    A'[k]  = windowed featT @ w0[k]  (PE, f16 in / fp32 psum), windows chosen
             per-core (th) so the program structure is th-independent.
    A' -> DRAM (flat rows) -> strided overlapping-window gather DMAs pack the
             needed (k,tau) rows densely into per-(d-group) stationary tiles
             [128, NTC, TM, 512] (DRAM is flat, so arbitrary row packing is
             free; compute engines cannot shift partitions by non-32).
    x2     = packed_A.T @ wsmp_packed   (TM=2 matmuls per (chunk, o-chunk))
    x3     = w1.T @ x2 (per chunk) -> written straight into the conv pad buf
    x4     = 3x3 conv (9 taps), out = sigmoid(w3.T x4 + b3).
  All 16-bit operands are fp16; PSUM accumulates fp32. Activations+bias+
  evictions are fused single ACT-engine ops (Lrelu/Sigmoid).
"""
import os
import sys

for _p in ("/opt/trn_rl_repo", "/root/.axon_site/_ro/trn_rl_repo"):
    if os.path.isdir(_p) and _p not in sys.path:
        sys.path.append(_p)

import numpy as np

import concourse.bass as bass
import concourse.tile as tile
from concourse import mybir
from concourse.bass_utils import run_bass_kernel_spmd
from concourse.tile_rust import add_dep_helper

T = 256
N = 10
D = 40
B = 4
C_IN = 256
DIM0 = 512
DIM1 = 128
TW = 130

TC = 18
NTC = (TW + TC - 1) // TC          # 8
DGS = (21, 19)
D0S = (0, 21)
CMAX = DGS[0] * TC                 # 378
SHIFTS = [27 - 4 * k for k in range(N)]
DCH = 3
NDCH = (D + DCH - 1) // DCH        # 14

F32 = mybir.dt.float32
F16 = mybir.dt.float16
LRELU = None  # resolved at build: mybir.ActivationFunctionType.Lrelu


def _legalize_waits(nc, limit=1):
    """Walrus build allows one embedded sync wait per real instruction;
    move the excess onto standalone NoOp wait-carriers."""
    for f in nc.m.functions:
        for bb in f.blocks:
            out = []
            changed = False
            for inst in bb.instructions:
                si = inst.sync_info
                ty = type(inst).__name__
                if (si and si.on_wait and len(si.on_wait) > limit
                        and ty not in ("InstEventSemaphore", "InstNoOp")):
                    keep = si.on_wait[-limit:]
                    for w in si.on_wait[:-limit]:
                        out.append(mybir.InstNoOp(
                            name=f"waitnop-{nc.next_id()}",
                            sync_info=mybir.SyncInfo(on_wait=[w], on_update=[]),
                            bass_nofuse=True,
                            engine=inst.engine,
                        ))
                    inst.sync_info = mybir.SyncInfo(
                        on_wait=keep, on_update=si.on_update)
                    changed = True
                out.append(inst)
            if changed:
                bb.instructions = out


def _try_two_bins(lens, cap=128):
    items = [(L, i) for i, L in enumerate(lens) if L > 0]
    R = sum(L for L, _ in items)
    if R > 2 * cap:
        return None
    # subset-sum: find subset with sum in [R-cap, cap]
    reach = {0: ()}
    for L, i in items:
        new = {}
        for s, sel in reach.items():
            if s + L <= cap and s + L not in reach:
                new[s + L] = sel + (i,)
        reach.update(new)
    best = None
    for s, sel in reach.items():
        if R - s <= cap and (best is None or s > best[0]):
            best = (s, sel)
    if best is None:
        return None
    sel = set(best[1])
    assign = {}
    off0 = off1 = 0
    for L, i in items:
        if i in sel:
            assign[i] = (0, off0)
            off0 += L
        else:
            assign[i] = (1, off1)
            off1 += L
    return 2, assign


def _ffd_bins(lens, cap=128):
    """Pack items into bins of `cap`. Tries an exact 2-bin split (padding an
    item by up to +3 rows to fix subset-sum parity), else FFD."""
    for extra_i in range(-1, len(lens)):
        for delta in ([0] if extra_i < 0 else [1, 2, 3]):
            ll = list(lens)
            if extra_i >= 0:
                if ll[extra_i] == 0:
                    continue
                ll[extra_i] += delta
            r = _try_two_bins(ll, cap)
            if r is not None:
                return r[0], r[1], ll
    order = sorted(range(len(lens)), key=lambda i: -lens[i])
    bins = []
    assign = {}
    for i in order:
        L = lens[i]
        if L == 0:
            continue
        placed = False
        for bi in range(len(bins)):
            if bins[bi] + L <= cap:
                assign[i] = (bi, bins[bi])
                bins[bi] += L
                placed = True
                break
        if not placed:
            bins.append(L)
            assign[i] = (len(bins) - 1, 0)
    return len(bins), assign, list(lens)


def build_structure(smp):
    """smp: (T, N, D, T) float32.  Returns the (th-independent) program
    structure plus per-th offsets for host data construction."""
    # per (dgi, k, th): linearized span (alo at tci=0, len) from actual data
    alo = {}
    ln = {}
    for dgi, (d0, dg) in enumerate(zip(D0S, DGS)):
        for k in range(N):
            sh = SHIFTS[k]
            for th in range(2):
                t0 = th * 128
                spans = []
                for tci in range(NTC):
                    ts = t0 - 1 + tci * TC
                    te = min(t0 - 1 + min(TW, (tci + 1) * TC), T)
                    tsc = max(ts, 0)
                    sub = smp[:, k, d0:d0 + dg, tsc:te]
                    nz = np.nonzero(np.abs(sub).max(axis=(1, 2)) > 0)[0]
                    spans.append((int(nz.min()), int(nz.max()) + 1)
                                 if len(nz) else None)
                valid = [(tci, s) for tci, s in enumerate(spans) if s]
                if not valid:
                    alo[(dgi, k, th)] = 0
                    ln[(dgi, k, th)] = 0
                    continue
                a = min(s[0] - tci * sh for tci, s in valid)
                L = max(s[1] - (a + tci * sh) for tci, s in valid)
                alo[(dgi, k, th)] = int(a)
                ln[(dgi, k, th)] = int(L)

    # per (k, th): window start; th-independent rel offsets + lens
    off = {}
    W = [0] * N
    rel = {}
    Lu = {}
    for k in range(N):
        sh = SHIFTS[k]
        for th in range(2):
            lo_cov = min(alo[(dgi, k, th)] + min(0, (NTC - 1) * sh)
                         for dgi in range(len(DGS)))
            off[(k, th)] = int(lo_cov)
        for dgi in range(len(DGS)):
            r = min(alo[(dgi, k, th)] - off[(k, th)] for th in range(2))
            L = max(alo[(dgi, k, th)] - off[(k, th)] + ln[(dgi, k, th)]
                    for th in range(2)) - r
            rel[(dgi, k)] = int(r)
            Lu[(dgi, k)] = int(L)
        W[k] = max(rel[(dgi, k)] + max(0, (NTC - 1) * sh) + Lu[(dgi, k)]
                   for dgi in range(len(DGS)))
        wmin = min(rel[(dgi, k)] + min(0, (NTC - 1) * sh)
                   for dgi in range(len(DGS)))
        assert wmin >= 0, (k, wmin)

    wstart = [int(x) for x in np.concatenate([[0], np.cumsum(W)])]
    wtot = int(wstart[-1])
    W = [int(x) for x in W]

    # bins per dgi; gather lengths (Lg) are filler-extended so each bin is
    # exactly 128 rows -> gather DMAs fully cover the packed tiles (no memset)
    TM = []
    binassign = []
    Lg = {}
    for dgi in range(len(DGS)):
        lens = [Lu[(dgi, k)] for k in range(N)]
        nb, assign, plens = _ffd_bins(lens)
        TM.append(nb)
        binassign.append(assign)
        for k in range(N):
            if plens[k] > 0:
                Lg[(dgi, k)] = int(plens[k])
                need = rel[(dgi, k)] + max(0, (NTC - 1) * SHIFTS[k]) + \
                    int(plens[k])
                W[k] = max(W[k], need)

    wstart = [int(x) for x in np.concatenate([[0], np.cumsum(W)])]
    wtot = int(wstart[-1])
    W = [int(x) for x in W]

    # A' pieces (per k, <=128 rows each)
    pieces = []
    for k in range(N):
        o = 0
        while o < W[k]:
            plen = min(128, W[k] - o)
            pieces.append((k, int(wstart[k] + o), o, plen))
            o += plen

    # verify coverage: every nonzero of every th is inside its segment
    for dgi, (d0, dg) in enumerate(zip(D0S, DGS)):
        for k in range(N):
            sh = SHIFTS[k]
            for th in range(2):
                t0 = th * 128
                for tci in range(NTC):
                    ts = t0 - 1 + tci * TC
                    te = min(t0 - 1 + min(TW, (tci + 1) * TC), T)
                    tsc = max(ts, 0)
                    sub = smp[:, k, d0:d0 + dg, tsc:te]
                    nz = np.nonzero(np.abs(sub).max(axis=(1, 2)) > 0)[0]
                    if not len(nz):
                        continue
                    lo = off[(k, th)] + rel[(dgi, k)] + tci * sh
                    assert lo <= nz.min() and nz.max() < lo + Lu[(dgi, k)], \
                        (dgi, k, th, tci, lo, nz.min(), nz.max())

    return dict(alo=alo, ln=ln, off=off, W=W, rel=rel, Lu=Lu, Lg=Lg,
                wstart=wstart, wtot=wtot, pieces=pieces, TM=TM,
                bins=binassign)


def _structure_key(S):
    return (tuple(S["W"]), tuple(S["TM"]),
            tuple(sorted((k, v) for k, v in S["rel"].items())),
            tuple(sorted((k, v) for k, v in S["Lu"].items())),
            tuple(sorted((k, v) for k, v in S["Lg"].items())),
            tuple(S["pieces"]),
            tuple(sorted((k, (v[0], v[1])) for k, v in S["bins"][0].items())),
            tuple(sorted((k, (v[0], v[1])) for k, v in S["bins"][1].items())))


def _build_program(S, zb=True, debug=False):
    lrelu = mybir.ActivationFunctionType.Lrelu
    sigm = mybir.ActivationFunctionType.Sigmoid
    copyf = mybir.ActivationFunctionType.Copy

    wtot = S["wtot"]
    TM = S["TM"]
    TB = (TM[0] + TM[1]) * NTC      # total moving bins
    nc = bass.Bass(trn_type="TRN2")

    fs_d = nc.dram_tensor("fs", [2, 128, wtot], F16, kind="ExternalInput")
    w0_d = nc.dram_tensor("w0p", [2, 128, N * DIM0], F16, kind="ExternalInput")
    wsmp_d = nc.dram_tensor("wsmp", [TB, 128, CMAX], F16, kind="ExternalInput")
    w1_d = nc.dram_tensor("w1p", [4, 128, DIM1], F16, kind="ExternalInput")
    w2_d = nc.dram_tensor("w2p", [9, 128, DIM1], F16, kind="ExternalInput")
    w3_d = nc.dram_tensor("w3p", [128, 1], F16, kind="ExternalInput")
    b0_d = nc.dram_tensor("b0", [4, 128, 1], F32, kind="ExternalInput")
    b1_d = nc.dram_tensor("b1", [128, 1], F32, kind="ExternalInput")
    b2_d = nc.dram_tensor("b2", [128, 1], F32, kind="ExternalInput")
    b3_d = nc.dram_tensor("b3", [1, 1], F32, kind="ExternalInput")
    afl_d = nc.dram_tensor("aflat", [len(S["pieces"]) * 128, DIM0], F16,
                           kind="Internal")
    out_d = nc.dram_tensor("out", [1, D * TW], F32, kind="ExternalOutput")
    if debug:
        dbg_x2 = nc.dram_tensor("dbg_x2", [4, 128, CMAX], F16,
                                kind="ExternalOutput")
        dbg_pad = nc.dram_tensor("dbg_pad", [128, (D + 2) * (TW + 2)], F16,
                                 kind="ExternalOutput")
        dbg_x4 = nc.dram_tensor("dbg_x4", [128, DCH * TW], F16,
                                kind="ExternalOutput")

    with tile.TileContext(nc) as tc:
        with (
            tc.tile_pool(name="inp", bufs=1) as inp,
            tc.tile_pool(name="apool", bufs=3) as apool,
            tc.tile_pool(name="pk", bufs=1) as pkp,
            tc.tile_pool(name="wst", bufs=3) as wst,
            tc.tile_pool(name="x2p", bufs=2) as x2p,
            tc.tile_pool(name="x3p", bufs=1) as x3p,
            tc.tile_pool(name="x4p", bufs=1) as x4p,
            tc.tile_pool(name="outp", bufs=1) as outp,
            tc.tile_pool(name="psA", bufs=2, space="PSUM") as psA,
            tc.tile_pool(name="psB", bufs=2, space="PSUM") as psB,
            tc.tile_pool(name="psC", bufs=2, space="PSUM") as psC,
        ):
            # ---------------- input DMAs ----------------
            fs_sb = []
            fs_dmas = []
            for c in range(2):
                t_ = inp.tile([128, wtot], F16, tag=f"fs{c}", name=f"fs{c}")
                fs_dmas.append(nc.sync.dma_start(t_[:], fs_d[c]))
                fs_sb.append(t_)
            w0_sb = []
            w0_dmas = {}
            for c in range(2):
                t_ = inp.tile([128, N * DIM0], F16, tag=f"w0{c}",
                              name=f"w0{c}")
                w0_sb.append(t_)
                eng = nc.sync if c == 0 else nc.scalar
                for k in range(N):
                    w0_dmas[(c, k)] = eng.dma_start(
                        t_[:, k * DIM0:(k + 1) * DIM0],
                        w0_d[c, :, k * DIM0:(k + 1) * DIM0])
            w1t = inp.tile([128, 4 * DIM1], F16, tag="w1", name="w1_sb")
            nc.sync.dma_start(w1t[:].rearrange("p (a b) -> p a b", a=4),
                              w1_d[:].transpose((1, 0, 2)))
            w2t = inp.tile([128, 9 * DIM1], F16, tag="w2", name="w2_sb")
            nc.sync.dma_start(w2t[:].rearrange("p (a b) -> p a b", a=9),
                              w2_d[:].transpose((1, 0, 2)))
            w3t = inp.tile([128, 1], F16, tag="w3", name="w3_sb")
            nc.sync.dma_start(w3t[:], w3_d[:])
            b0t = inp.tile([128, 4], F32, tag="b0", name="b0_sb")
            nc.sync.dma_start(b0t[:].rearrange("p (a b) -> p a b", b=1),
                              b0_d[:].transpose((1, 0, 2)))
            b1t = inp.tile([128, 1], F32, tag="b1", name="b1_sb")
            nc.sync.dma_start(b1t[:], b1_d[:])
            b2t = inp.tile([128, 1], F32, tag="b2", name="b2_sb")
            nc.sync.dma_start(b2t[:], b2_d[:])
            b3t = inp.tile([1, 1], F32, tag="b3", name="b3_sb")
            nc.sync.dma_start(b3t[:], b3_d[:])

            # packed stationary tiles (one per d-group), zeroed once (f32
            # view: 2x faster) so bin gap rows can never be NaN
            pk = []
            for dgi in range(2):
                t_ = pkp.tile([128, NTC, TM[dgi], DIM0], F16,
                              tag=f"pk{dgi}", name=f"pk{dgi}")
                nc.vector.memset(
                    t_[:].rearrange("p a b f -> p (a b f)").bitcast(F32), 0.0)
                pk.append(t_)
            # conv pad buffer, zeroed once (f32 view: 2x faster memset)
            pad = x3p.tile([128, D + 2, TW + 2], F16, tag="pad", name="pad")
            nc.vector.memset(
                pad[:].rearrange("p a b -> p (a b)").bitcast(F32), 0.0)

            # warm-up accumulation group riding on input DMAs (keeps the PE
            # HAM window busy through the DMA prologue)
            warm = psC.tile([1, 4], F32, tag="c", name="warm_ps")
            nc.tensor.matmul(warm[:], fs_sb[0][:, 0:1], fs_sb[0][:, 0:4],
                             start=True, stop=False)
            nc.tensor.matmul(warm[:], fs_sb[1][:, 0:1], fs_sb[1][:, 0:4],
                             start=False, stop=False)
            for c in range(2):
                for k in (0, 3, 6, 9):
                    nc.tensor.matmul(
                        warm[:], w0_sb[c][:, k * DIM0:k * DIM0 + 1],
                        w0_sb[c][:, k * DIM0:k * DIM0 + 4],
                        start=False, stop=(c == 1 and k == 9))

            # ---------------- stage A -> one big SBUF tile + 2 dump DMAs --
            npc = len(S["pieces"])
            aev = apool.tile([128, npc * DIM0], F16, tag="aev", name="aev")
            phalf = (npc + 1) // 2
            for pi, (k, absrow, relrow, plen) in enumerate(S["pieces"]):
                ps = psA.tile([128, DIM0], F32, tag="a", name=f"psa{pi}")
                for c in range(2):
                    nc.tensor.matmul(
                        ps[0:plen, :],
                        fs_sb[c][:, S["wstart"][k] + relrow:
                                 S["wstart"][k] + relrow + plen],
                        w0_sb[c][:, k * DIM0:(k + 1) * DIM0],
                        start=(c == 0), stop=(c == 1),
                    )
                nc.vector.tensor_copy(
                    aev[0:plen, pi * DIM0:(pi + 1) * DIM0], ps[0:plen, :])
            # dump halves: pieces are 128-row aligned in afl (piece pi ->
            # afl rows [pstart(pi)*... ]) -- afl layout is piece-major
            dump_dmas = []
            qs = sorted({0, npc // 4, npc // 2, (3 * npc) // 4, npc})
            for h, (p0, p1) in enumerate(zip(qs, qs[1:])):
                src = aev[:, p0 * DIM0:p1 * DIM0].rearrange(
                    "p (a f) -> p a f", f=DIM0)
                dst = afl_d[p0 * 128:p1 * 128, :].rearrange(
                    "(a p) f -> p a f", p=128)
                eng = nc.sync if h % 2 == 0 else nc.scalar
                dump_dmas.append((p0, p1, eng.dma_start(dst, src)))

            # ---------------- gathers: aflat -> packed stationaries -------
            # afl rows are PIECE-padded: window k starts at piece boundary
            pstart = {}
            for pi, (k, absrow, relrow, plen) in enumerate(S["pieces"]):
                if relrow == 0:
                    pstart[k] = pi * 128
            geng = [nc.gpsimd]
            gi = 0
            for dgi in range(2):
                for k in range(N):
                    L = S["Lg"].get((dgi, k), 0)
                    if L == 0:
                        continue
                    m, dstoff = S["bins"][dgi][k]
                    sh = SHIFTS[k]
                    base = pstart[k] + S["rel"][(dgi, k)]
                    src = afl_d[:, :].copy()
                    src.ap = mybir.VecI64Pair(
                        [[DIM0, L], [sh * DIM0, NTC], [1, DIM0]])
                    src.offset = base * DIM0
                    dst = pk[dgi][dstoff:dstoff + L, :, m, :]
                    gd = geng[gi % len(geng)].dma_start(dst, src)
                    gi += 1
                    lo = base + min(0, (NTC - 1) * sh)
                    hi = base + max(0, (NTC - 1) * sh) + L
                    for (p0, p1, dd) in dump_dmas:
                        if p0 * 128 < hi and lo < p1 * 128:
                            add_dep_helper(gd.ins, dd.ins,
                                           reason="aflat roundtrip order")

            # ---------------- stages B + C per chunk ----------------
            binbase = [0, TM[0] * NTC]
            for dgi in range(2):
                d0, dg = D0S[dgi], DGS[dgi]
                tm = TM[dgi]
                for tci in range(NTC):
                    tcw = min(TC, TW - tci * TC)
                    cols = dg * tcw
                    bb = binbase[dgi] + tci * tm
                    wt = wst.tile([128, tm, CMAX], F16, tag=f"ws{dgi}",
                                  name=f"ws{dgi}_{tci}")
                    nc.sync.dma_start(
                        wt[:], wsmp_d[bb:bb + tm].transpose((1, 0, 2)))
                    if dgi == 0 and tci < 4:
                        # keep the PE HAM window warm across the gather gap
                        wm = psC.tile([1, 4], F32, tag="c",
                                      name=f"warmb{tci}")
                        nc.tensor.matmul(wm[:], wt[:, 0, 0:1],
                                         wt[:, 0, 0:4],
                                         start=True, stop=True)
                    x2c = [None] * 4
                    for g in range(2):
                        accs = []
                        for oo in range(2):
                            o = 2 * g + oo
                            acc = psB.tile([128, CMAX], F32, tag=f"b{oo}",
                                           name=f"psb{dgi}_{tci}_{o}")
                            accs.append(acc)
                            for m in range(tm):
                                nc.tensor.matmul(
                                    acc[:, 0:cols],
                                    pk[dgi][:, tci, m,
                                            o * 128:(o + 1) * 128],
                                    wt[:, m, 0:cols],
                                    start=(m == 0), stop=(m == tm - 1),
                                )
                        for oo in range(2):
                            o = 2 * g + oo
                            yt = x2p.tile([128, CMAX], F16, tag=f"x2{o}",
                                          name=f"x2_{dgi}_{tci}_{o}")
                            if oo == 0 or not zb:
                                nc.scalar.activation(
                                    yt[:, 0:cols], accs[oo][:, 0:cols],
                                    lrelu, bias=b0t[:, o:o + 1], scale=1.0)
                            else:
                                # DVE path (bias==0): copy-convert + leaky
                                nc.vector.tensor_copy(yt[:, 0:cols],
                                                      accs[oo][:, 0:cols])
                                nc.vector.scalar_tensor_tensor(
                                    yt[:, 0:cols], yt[:, 0:cols], 0.01,
                                    yt[:, 0:cols], mybir.AluOpType.mult,
                                    mybir.AluOpType.max)
                            x2c[o] = yt
                            if debug and dgi == 0 and tci == 2:
                                nc.sync.dma_start(
                                    dbg_x2[o, :, 0:cols],
                                    x2c[o][:, 0:cols])
                    psc = psC.tile([128, CMAX], F32, tag="c",
                                   name=f"psc{dgi}_{tci}")
                    for o in range(4):
                        nc.tensor.matmul(
                            psc[:, 0:cols],
                            w1t[:, o * DIM1:(o + 1) * DIM1],
                            x2c[o][:, 0:cols],
                            start=(o == 0), stop=(o == 3))
                    nc.scalar.activation(
                        pad[:, 1 + d0:1 + d0 + dg,
                            1 + tci * TC:1 + tci * TC + tcw],
                        psc[:, 0:cols].rearrange("p (a b) -> p a b", a=dg),
                        lrelu, bias=b1t[:], scale=1.0)

            # ---------------- stage D (3x3 conv) + E ----------------
            if debug:
                nc.sync.dma_start(
                    dbg_pad[:], pad[:].rearrange("p a b -> p (a b)"))
            out_sb = outp.tile([1, D * TW], F32, tag="os", name="out_sb")
            x4cs = [None] * NDCH

            def stage_e(dc):
                d0 = dc * DCH
                fw = min(DCH, D - d0) * TW
                pse = psC.tile([1, DCH * TW], F32, tag="c", name=f"pse{dc}")
                nc.tensor.matmul(pse[:, 0:fw], w3t[:], x4cs[dc][:, 0:fw],
                                 start=True, stop=True)
                nc.scalar.activation(
                    out_sb[:, d0 * TW:d0 * TW + fw], pse[:, 0:fw],
                    sigm, bias=b3t[:], scale=1.0)

            # all conv chunks first (Lrelu era), then all sigmoids: avoids
            # ACT function-table thrash from Lrelu/Sigmoid interleaving
            for dc in range(NDCH):
                d0 = dc * DCH
                nd = min(DCH, D - d0)
                fw = nd * TW
                psd = psA.tile([128, DCH * TW], F32, tag="a", name=f"psd{dc}")
                for j in range(9):
                    dy, dx = j // 3, j % 3
                    nc.tensor.matmul(
                        psd[:, 0:fw],
                        w2t[:, j * DIM1:(j + 1) * DIM1],
                        pad[:, d0 + dy:d0 + dy + nd, dx:dx + TW],
                        start=(j == 0), stop=(j == 8),
                    )
                x4c = x4p.tile([128, DCH * TW], F16, tag=f"x4_{dc}",
                               name=f"x4_{dc}")
                nc.scalar.activation(x4c[:, 0:fw], psd[:, 0:fw], lrelu,
                                     bias=b2t[:], scale=1.0)
                x4cs[dc] = x4c
                if debug and dc == 0:
                    nc.sync.dma_start(dbg_x4[:], x4c[:])
            for dc in range(NDCH):
                stage_e(dc)
            nc.scalar.dma_start(out_d[:], out_sb[:])
    _legalize_waits(nc)
    return nc


_PROGRAM = None
_STRUCT = None


def _get_structure(smp):
    global _STRUCT
    if _STRUCT is None:
        _STRUCT = build_structure(smp)
    return _STRUCT


def _get_program(S, zb=True, debug=False):
    global _PROGRAM
    key = (_structure_key(S), zb, debug)
    if _PROGRAM is None or _PROGRAM[0] != key:
        _PROGRAM = (key, _build_program(S, zb=zb, debug=debug))
    return _PROGRAM[1]


def _host_data(S, feature, smp, w0, b0, w1, b1, w2, b2, w3, b3):
    """Build per-core input maps."""
    wtot = S["wtot"]
    TM = S["TM"]
    TB = (TM[0] + TM[1]) * NTC

    w0p = np.ascontiguousarray(
        np.asarray(w0, np.float32).transpose(1, 2, 0)    # (C, N, DIM0)
        .reshape(2, 128, N * DIM0)).astype(np.float16)
    w1p = np.ascontiguousarray(
        np.asarray(w1, np.float32).T.reshape(4, 128, DIM1)).astype(np.float16)
    w2p = np.ascontiguousarray(
        np.asarray(w2, np.float32).transpose(2, 3, 1, 0).reshape(
            9, DIM1, DIM1)).astype(np.float16)
    w3p = np.ascontiguousarray(
        np.asarray(w3, np.float32).T).astype(np.float16)    # (128, 1)
    b0p = np.ascontiguousarray(
        np.asarray(b0, np.float32).reshape(4, 128, 1))
    b1p = np.asarray(b1, np.float32).reshape(128, 1)
    b2p = np.asarray(b2, np.float32).reshape(128, 1)
    b3p = np.asarray(b3, np.float32).reshape(1, 1)

    feature = np.asarray(feature, np.float32)

    # per-th packed wsmp
    wsmp_th = []
    for th in range(2):
        t0 = th * 128
        ws = np.zeros((TB, 128, CMAX), np.float32)
        binbase = [0, TM[0] * NTC]
        for dgi, (d0, dg) in enumerate(zip(D0S, DGS)):
            for k in range(N):
                L = S["Lu"][(dgi, k)]
                if L == 0:
                    continue
                m, dstoff = S["bins"][dgi][k]
                sh = SHIFTS[k]
                sub = smp[:, k, d0:d0 + dg, :]          # (T, dg, T)
                for tci in range(NTC):
                    tcw = min(TC, TW - tci * TC)
                    lo = S["off"][(k, th)] + S["rel"][(dgi, k)] + tci * sh
                    taus = lo + np.arange(L)
                    tmask = (taus >= 0) & (taus < T)
                    tcl = np.clip(taus, 0, T - 1)
                    tpos = t0 - 1 + tci * TC + np.arange(tcw)
                    pmask = (tpos >= 0) & (tpos < T)
                    tpl = np.clip(tpos, 0, T - 1)
                    blk = sub[tcl][:, :, tpl]            # (L, dg, tcw)
                    blk = blk * tmask[:, None, None] * pmask[None, None, :]
                    ws[binbase[dgi] + tci * TM[dgi] + m,
                       dstoff:dstoff + L, 0:dg * tcw] = blk.reshape(L, -1)
        wsmp_th.append(ws.astype(np.float16))

    in_maps = []
    for core in range(8):
        b, th = core // 2, core % 2
        fs = np.zeros((C_IN, wtot), np.float32)
        for k in range(N):
            ofk = S["off"][(k, th)]
            u0 = max(0, -ofk)
            u1 = min(S["W"][k], T - ofk)
            if u1 > u0:
                fs[:, S["wstart"][k] + u0:S["wstart"][k] + u1] = \
                    feature[b][:, ofk + u0:ofk + u1]
        in_maps.append({
            "fs": np.ascontiguousarray(
                fs.reshape(2, 128, wtot)).astype(np.float16),
            "w0p": w0p,
            "wsmp": wsmp_th[th],
            "w1p": w1p,
            "w2p": w2p,
            "w3p": w3p,
            "b0": b0p,
            "b1": b1p,
            "b2": b2p,
            "b3": b3p,
        })
    return in_maps


def kernel(feature, smp_weight, w0, b0, w1, b1, w2, b2, w3, b3,
           _trace=False, _debug=False):
    smp = np.asarray(smp_weight, np.float32).reshape(T, N, D, T)
    S = _get_structure(smp)
    zb = all(float(np.abs(np.asarray(x)).max()) == 0.0 for x in (b0, b1))
    nc = _get_program(S, zb=zb, debug=_debug)
    in_maps = _host_data(S, feature, smp, w0, b0, w1, b1, w2, b2, w3, b3)
    res = run_bass_kernel_spmd(nc, in_maps, core_ids=list(range(8)),
                               trace=_trace)
    out = np.empty((B, D, T), dtype=np.float32)
    for core in range(8):
        b, th = core // 2, core % 2
        full = res.results[core]["out"].reshape(D, TW)
        out[b, :, th * 128:(th + 1) * 128] = full[:, 1:TW - 1]
    if _trace or _debug:
        return out, res
    return out


# revision 3
# speedup vs baseline: 1.0940x; 1.0416x over previous
"""Trainium2 Bass kernel for nn_BoundaryModule_38422777430159 (v2).

Reference (B=4, C=256, T=256, N=10, D=40, DIM0=512, DIM1=128):
  x1 = sample(feature)            # (B,C,N,D,T) via (T, N*D*T) smp matmul
  x2 = leaky(einsum('bcndt,ocn->bodt', x1, w0) + b0)
  x3 = leaky(w1 @ x2 + b1)
  x4 = leaky(conv3x3(x3, w2) + b2)
  out = sigmoid(w3 @ x4 + b3)     # (B, D, T)

v2 strategy (8 cores SPMD, core = (b, t-half), TW=130 incl halo):
  The sampling matrix columns (k,d,t) have tau-support that is an interval
  moving AFFINELY in t with integer stride shift_k = 27-4k per Tc=18 t-chunk.
  Per core:
    A'[k]  = windowed featT @ w0[k]  (PE, f16 in / fp32 psum), windows chosen
             per-core (th) so the program structure is th-independent.
    A' -> DRAM (flat rows) -> strided overlapping-window gather DMAs pack the
             needed (k,tau) rows densely into per-(d-group) stationary tiles
             [128, NTC, TM, 512] (DRAM is flat, so arbitrary row packing is
             free; compute engines cannot shift partitions by non-32).
    x2     = packed_A.T @ wsmp_packed   (TM=2 matmuls per (chunk, o-chunk))
    x3     = w1.T @ x2 (per chunk) -> written straight into the conv pad buf
    x4     = 3x3 conv (9 taps), out = sigmoid(w3.T x4 + b3).
  All 16-bit operands are fp16; PSUM accumulates fp32. Activations+bias+
  evictions are fused single ACT-engine ops (Lrelu/Sigmoid).
"""
import os
import sys

for _p in ("/opt/trn_rl_repo", "/root/.axon_site/_ro/trn_rl_repo"):
    if os.path.isdir(_p) and _p not in sys.path:
        sys.path.append(_p)

import numpy as np

import concourse.bass as bass
import concourse.tile as tile
from concourse import mybir
from concourse.bass_utils import run_bass_kernel_spmd
from concourse.tile_rust import add_dep_helper

T = 256
N = 10
D = 40
B = 4
C_IN = 256
DIM0 = 512
DIM1 = 128
TW = 130

TC = 18
NTC = (TW + TC - 1) // TC          # 8
DGS = (21, 19)
D0S = (0, 21)
CMAX = DGS[0] * TC                 # 378
SHIFTS = [27 - 4 * k for k in range(N)]
DCH = 3
NDCH = (D + DCH - 1) // DCH        # 14

F32 = mybir.dt.float32
F16 = mybir.dt.float16
LRELU = None  # resolved at build: mybir.ActivationFunctionType.Lrelu


def _legalize_waits(nc, limit=1):
    """Walrus build allows one embedded sync wait per real instruction;
    move the excess onto standalone NoOp wait-carriers."""
    for f in nc.m.functions:
        for bb in f.blocks:
            out = []
            changed = False
            for inst in bb.instructions:
                si = inst.sync_info
                ty = type(inst).__name__
                if (si and si.on_wait and len(si.on_wait) > limit
                        and ty not in ("InstEventSemaphore", "InstNoOp")):
                    keep = si.on_wait[-limit:]
                    for w in si.on_wait[:-limit]:
                        out.append(mybir.InstNoOp(
                            name=f"waitnop-{nc.next_id()}",
                            sync_info=mybir.SyncInfo(on_wait=[w], on_update=[]),
                            bass_nofuse=True,
                            engine=inst.engine,
                        ))
                    inst.sync_info = mybir.SyncInfo(
                        on_wait=keep, on_update=si.on_update)
                    changed = True
                out.append(inst)
            if changed:
                bb.instructions = out


def _try_two_bins(lens, cap=128):
    items = [(L, i) for i, L in enumerate(lens) if L > 0]
    R = sum(L for L, _ in items)
    if R > 2 * cap:
        return None
    # subset-sum: find subset with sum in [R-cap, cap]
    reach = {0: ()}
    for L, i in items:
        new = {}
        for s, sel in reach.items():
            if s + L <= cap and s + L not in reach:
                new[s + L] = sel + (i,)
        reach.update(new)
    best = None
    for s, sel in reach.items():
        if R - s <= cap and (best is None or s > best[0]):
            best = (s, sel)
    if best is None:
        return None
    sel = set(best[1])
    assign = {}
    off0 = off1 = 0
    for L, i in items:
        if i in sel:
            assign[i] = (0, off0)
            off0 += L
        else:
            assign[i] = (1, off1)
            off1 += L
    return 2, assign


def _ffd_bins(lens, cap=128):
    """Pack items into bins of `cap`. Tries an exact 2-bin split (padding an
    item by up to +3 rows to fix subset-sum parity), else FFD."""
    for extra_i in range(-1, len(lens)):
        for delta in ([0] if extra_i < 0 else [1, 2, 3]):
            ll = list(lens)
            if extra_i >= 0:
                if ll[extra_i] == 0:
                    continue
                ll[extra_i] += delta
            r = _try_two_bins(ll, cap)
            if r is not None:
                return r[0], r[1], ll
    order = sorted(range(len(lens)), key=lambda i: -lens[i])
    bins = []
    assign = {}
    for i in order:
        L = lens[i]
        if L == 0:
            continue
        placed = False
        for bi in range(len(bins)):
            if bins[bi] + L <= cap:
                assign[i] = (bi, bins[bi])
                bins[bi] += L
                placed = True
                break
        if not placed:
            bins.append(L)
            assign[i] = (len(bins) - 1, 0)
    return len(bins), assign, list(lens)


def build_structure(smp):
    """smp: (T, N, D, T) float32.  Returns the (th-independent) program
    structure plus per-th offsets for host data construction."""
    # per (dgi, k, th): linearized span (alo at tci=0, len) from actual data
    alo = {}
    ln = {}
    for dgi, (d0, dg) in enumerate(zip(D0S, DGS)):
        for k in range(N):
            sh = SHIFTS[k]
            for th in range(2):
                t0 = th * 128
                spans = []
                for tci in range(NTC):
                    ts = t0 - 1 + tci * TC
                    te = min(t0 - 1 + min(TW, (tci + 1) * TC), T)
                    tsc = max(ts, 0)
                    sub = smp[:, k, d0:d0 + dg, tsc:te]
                    nz = np.nonzero(np.abs(sub).max(axis=(1, 2)) > 0)[0]
                    spans.append((int(nz.min()), int(nz.max()) + 1)
                                 if len(nz) else None)
                valid = [(tci, s) for tci, s in enumerate(spans) if s]
                if not valid:
                    alo[(dgi, k, th)] = 0
                    ln[(dgi, k, th)] = 0
                    continue
                a = min(s[0] - tci * sh for tci, s in valid)
                L = max(s[1] - (a + tci * sh) for tci, s in valid)
                alo[(dgi, k, th)] = int(a)
                ln[(dgi, k, th)] = int(L)

    # Window selection: a window is shared across d-groups when the
    # th-unified union costs little; otherwise split per (k, dgi) (clipping
    # asymmetry between t-halves can inflate the shared union a lot).
    NDG = len(DGS)
    wids = []            # list of (k, dgi-or-None)
    widx = {}            # (dgi, k) -> window index
    off = {}             # (wi, th) -> absolute tau of window start
    rel = {}
    Lu = {}
    W = []
    for k in range(N):
        sh = SHIFTS[k]
        sh_lo = min(0, (NTC - 1) * sh)
        # shared-window candidate
        offs = {th: min(alo[(dgi, k, th)] + sh_lo for dgi in range(NDG))
                for th in range(2)}
        sh_rel = {}
        sh_Lu = {}
        for dgi in range(NDG):
            r = min(alo[(dgi, k, th)] - offs[th] for th in range(2))
            L = max(alo[(dgi, k, th)] - offs[th] + ln[(dgi, k, th)]
                    for th in range(2)) - r
            sh_rel[dgi] = int(r)
            sh_Lu[dgi] = int(L)
        split_Lu = {dgi: max(ln[(dgi, k, th)] for th in range(2))
                    for dgi in range(NDG)}
        if sum(sh_Lu.values()) - sum(split_Lu.values()) > 6:
            for dgi in range(NDG):
                wi = len(wids)
                wids.append((k, dgi))
                widx[(dgi, k)] = wi
                for th in range(2):
                    off[(wi, th)] = int(alo[(dgi, k, th)] + sh_lo)
                rel[(dgi, k)] = int(-sh_lo)
                Lu[(dgi, k)] = int(split_Lu[dgi])
                W.append(int(rel[(dgi, k)] + max(0, (NTC - 1) * sh)
                             + Lu[(dgi, k)]))
        else:
            wi = len(wids)
            wids.append((k, None))
            for dgi in range(NDG):
                widx[(dgi, k)] = wi
                rel[(dgi, k)] = sh_rel[dgi]
                Lu[(dgi, k)] = sh_Lu[dgi]
            for th in range(2):
                off[(wi, th)] = int(offs[th])
            W.append(int(max(rel[(dgi, k)] + max(0, (NTC - 1) * sh)
                             + Lu[(dgi, k)] for dgi in range(NDG))))
            wmin = min(rel[(dgi, k)] + sh_lo for dgi in range(NDG))
            assert wmin >= 0, (k, wmin)

    # bins per dgi; gather lengths (Lg) are filler-extended so each bin is
    # exactly 128 rows -> gather DMAs fully cover the packed tiles (no memset)
    TM = []
    binassign = []
    Lg = {}
    for dgi in range(len(DGS)):
        lens = [Lu[(dgi, k)] for k in range(N)]
        nb, assign, plens = _ffd_bins(lens)
        TM.append(nb)
        binassign.append(assign)
        for k in range(N):
            if plens[k] > 0:
                Lg[(dgi, k)] = int(plens[k])
                need = rel[(dgi, k)] + max(0, (NTC - 1) * SHIFTS[k]) + \
                    int(plens[k])
                wi = widx[(dgi, k)]
                W[wi] = max(W[wi], need)

    wstart = [int(x) for x in np.concatenate([[0], np.cumsum(W)])]
    wtot = int(wstart[-1])
    W = [int(x) for x in W]

    # A' pieces (per window, <=128 rows each)
    pieces = []
    for wi, (k, _dg) in enumerate(wids):
        o = 0
        while o < W[wi]:
            plen = min(128, W[wi] - o)
            pieces.append((k, wi, int(wstart[wi] + o), o, plen))
            o += plen

    # verify coverage: every nonzero of every th is inside its segment
    for dgi, (d0, dg) in enumerate(zip(D0S, DGS)):
        for k in range(N):
            sh = SHIFTS[k]
            for th in range(2):
                t0 = th * 128
                for tci in range(NTC):
                    ts = t0 - 1 + tci * TC
                    te = min(t0 - 1 + min(TW, (tci + 1) * TC), T)
                    tsc = max(ts, 0)
                    sub = smp[:, k, d0:d0 + dg, tsc:te]
                    nz = np.nonzero(np.abs(sub).max(axis=(1, 2)) > 0)[0]
                    if not len(nz):
                        continue
                    lo = off[(widx[(dgi, k)], th)] + rel[(dgi, k)] + tci * sh
                    assert lo <= nz.min() and nz.max() < lo + Lu[(dgi, k)], \
                        (dgi, k, th, tci, lo, nz.min(), nz.max())

    return dict(alo=alo, ln=ln, off=off, W=W, rel=rel, Lu=Lu, Lg=Lg,
                wstart=wstart, wtot=wtot, pieces=pieces, TM=TM,
                bins=binassign, wids=wids, widx=widx)


def _structure_key(S):
    return (tuple(S["W"]), tuple(S["TM"]), tuple(S["wids"]),
            tuple(sorted((k, v) for k, v in S["rel"].items())),
            tuple(sorted((k, v) for k, v in S["Lu"].items())),
            tuple(sorted((k, v) for k, v in S["Lg"].items())),
            tuple(S["pieces"]),
            tuple(sorted((k, (v[0], v[1])) for k, v in S["bins"][0].items())),
            tuple(sorted((k, (v[0], v[1])) for k, v in S["bins"][1].items())))


def _build_program(S, zb=True, debug=False):
    lrelu = mybir.ActivationFunctionType.Lrelu
    sigm = mybir.ActivationFunctionType.Sigmoid
    copyf = mybir.ActivationFunctionType.Copy

    wtot = S["wtot"]
    TM = S["TM"]
    TB = (TM[0] + TM[1]) * NTC      # total moving bins
    nc = bass.Bass(trn_type="TRN2")

    fs_d = nc.dram_tensor("fs", [2, 128, wtot], F16, kind="ExternalInput")
    w0_d = nc.dram_tensor("w0p", [2, 128, N * DIM0], F16, kind="ExternalInput")
    wsmp_d = nc.dram_tensor("wsmp", [TB, 128, CMAX], F16, kind="ExternalInput")
    w1_d = nc.dram_tensor("w1p", [4, 128, DIM1], F16, kind="ExternalInput")
    w2_d = nc.dram_tensor("w2p", [9, 128, DIM1], F16, kind="ExternalInput")
    w3_d = nc.dram_tensor("w3p", [128, 1], F16, kind="ExternalInput")
    b0_d = nc.dram_tensor("b0", [4, 128, 1], F32, kind="ExternalInput")
    b1_d = nc.dram_tensor("b1", [128, 1], F32, kind="ExternalInput")
    b2_d = nc.dram_tensor("b2", [128, 1], F32, kind="ExternalInput")
    b3_d = nc.dram_tensor("b3", [1, 1], F32, kind="ExternalInput")
    afl_d = nc.dram_tensor("aflat", [len(S["pieces"]) * 128, DIM0], F16,
                           kind="Internal")
    out_d = nc.dram_tensor("out", [1, D * TW], F32, kind="ExternalOutput")
    if debug:
        dbg_x2 = nc.dram_tensor("dbg_x2", [4, 128, CMAX], F16,
                                kind="ExternalOutput")
        dbg_pad = nc.dram_tensor("dbg_pad", [128, (D + 2) * (TW + 2)], F16,
                                 kind="ExternalOutput")
        dbg_x4 = nc.dram_tensor("dbg_x4", [128, DCH * TW], F16,
                                kind="ExternalOutput")

    with tile.TileContext(nc) as tc:
        with (
            tc.tile_pool(name="inp", bufs=1) as inp,
            tc.tile_pool(name="apool", bufs=3) as apool,
            tc.tile_pool(name="pk", bufs=1) as pkp,
            tc.tile_pool(name="wst", bufs=3) as wst,
            tc.tile_pool(name="x2p", bufs=2) as x2p,
            tc.tile_pool(name="x3p", bufs=1) as x3p,
            tc.tile_pool(name="x4p", bufs=1) as x4p,
            tc.tile_pool(name="outp", bufs=1) as outp,
            tc.tile_pool(name="psA", bufs=2, space="PSUM") as psA,
            tc.tile_pool(name="psB", bufs=2, space="PSUM") as psB,
            tc.tile_pool(name="psC", bufs=2, space="PSUM") as psC,
        ):
            # ---------------- input DMAs ----------------
            fs_sb = []
            fs_dmas = []
            for c in range(2):
                t_ = inp.tile([128, wtot], F16, tag=f"fs{c}", name=f"fs{c}")
                fs_dmas.append(nc.sync.dma_start(t_[:], fs_d[c]))
                fs_sb.append(t_)
            w0_sb = []
            w0_dmas = {}
            for c in range(2):
                t_ = inp.tile([128, N * DIM0], F16, tag=f"w0{c}",
                              name=f"w0{c}")
                w0_sb.append(t_)
                eng = nc.sync if c == 0 else nc.scalar
                for k in range(N):
                    w0_dmas[(c, k)] = eng.dma_start(
                        t_[:, k * DIM0:(k + 1) * DIM0],
                        w0_d[c, :, k * DIM0:(k + 1) * DIM0])
            w1t = inp.tile([128, 4 * DIM1], F16, tag="w1", name="w1_sb")
            nc.sync.dma_start(w1t[:].rearrange("p (a b) -> p a b", a=4),
                              w1_d[:].transpose((1, 0, 2)))
            w2t = inp.tile([128, 9 * DIM1], F16, tag="w2", name="w2_sb")
            nc.sync.dma_start(w2t[:].rearrange("p (a b) -> p a b", a=9),
                              w2_d[:].transpose((1, 0, 2)))
            w3t = inp.tile([128, 1], F16, tag="w3", name="w3_sb")
            nc.sync.dma_start(w3t[:], w3_d[:])
            b0t = inp.tile([128, 4], F32, tag="b0", name="b0_sb")
            nc.sync.dma_start(b0t[:].rearrange("p (a b) -> p a b", b=1),
                              b0_d[:].transpose((1, 0, 2)))
            b1t = inp.tile([128, 1], F32, tag="b1", name="b1_sb")
            nc.sync.dma_start(b1t[:], b1_d[:])
            b2t = inp.tile([128, 1], F32, tag="b2", name="b2_sb")
            nc.sync.dma_start(b2t[:], b2_d[:])
            b3t = inp.tile([1, 1], F32, tag="b3", name="b3_sb")
            nc.sync.dma_start(b3t[:], b3_d[:])

            # packed stationary tiles (one per d-group), zeroed once (f32
            # view: 2x faster) so bin gap rows can never be NaN
            pk = []
            for dgi in range(2):
                t_ = pkp.tile([128, NTC, TM[dgi], DIM0], F16,
                              tag=f"pk{dgi}", name=f"pk{dgi}")
                nc.vector.memset(
                    t_[:].rearrange("p a b f -> p (a b f)").bitcast(F32), 0.0)
                pk.append(t_)
            # conv pad buffer, zeroed once (f32 view: 2x faster memset)
            pad = x3p.tile([128, D + 2, TW + 2], F16, tag="pad", name="pad")
            nc.vector.memset(
                pad[:].rearrange("p a b -> p (a b)").bitcast(F32), 0.0)

            # warm-up accumulation group riding on input DMAs (keeps the PE
            # HAM window busy through the DMA prologue)
            warm = psC.tile([1, 4], F32, tag="c", name="warm_ps")
            nc.tensor.matmul(warm[:], fs_sb[0][:, 0:1], fs_sb[0][:, 0:4],
                             start=True, stop=False)
            nc.tensor.matmul(warm[:], fs_sb[1][:, 0:1], fs_sb[1][:, 0:4],
                             start=False, stop=False)
            for c in range(2):
                for k in (0, 3, 6, 9):
                    nc.tensor.matmul(
                        warm[:], w0_sb[c][:, k * DIM0:k * DIM0 + 1],
                        w0_sb[c][:, k * DIM0:k * DIM0 + 4],
                        start=False, stop=(c == 1 and k == 9))

            # ---------------- stage A -> one big SBUF tile + 2 dump DMAs --
            npc = len(S["pieces"])
            aev = apool.tile([128, npc * DIM0], F16, tag="aev", name="aev")
            phalf = (npc + 1) // 2
            for pi, (k, wi, absrow, relrow, plen) in enumerate(S["pieces"]):
                ps = psA.tile([128, DIM0], F32, tag="a", name=f"psa{pi}")
                for c in range(2):
                    nc.tensor.matmul(
                        ps[0:plen, :],
                        fs_sb[c][:, S["wstart"][wi] + relrow:
                                 S["wstart"][wi] + relrow + plen],
                        w0_sb[c][:, k * DIM0:(k + 1) * DIM0],
                        start=(c == 0), stop=(c == 1),
                    )
                nc.vector.tensor_copy(
                    aev[0:plen, pi * DIM0:(pi + 1) * DIM0], ps[0:plen, :])
            # dump halves: pieces are 128-row aligned in afl (piece pi ->
            # afl rows [pstart(pi)*... ]) -- afl layout is piece-major
            dump_dmas = []
            qs = sorted({0, npc // 4, npc // 2, (3 * npc) // 4, npc})
            for h, (p0, p1) in enumerate(zip(qs, qs[1:])):
                src = aev[:, p0 * DIM0:p1 * DIM0].rearrange(
                    "p (a f) -> p a f", f=DIM0)
                dst = afl_d[p0 * 128:p1 * 128, :].rearrange(
                    "(a p) f -> p a f", p=128)
                eng = nc.sync if h % 2 == 0 else nc.scalar
                dump_dmas.append((p0, p1, eng.dma_start(dst, src)))

            # ---------------- gathers: aflat -> packed stationaries -------
            # afl rows are PIECE-padded: window k starts at piece boundary
            pstart = {}
            for pi, (k, wi, absrow, relrow, plen) in enumerate(S["pieces"]):
                if relrow == 0:
                    pstart[wi] = pi * 128
            geng = [nc.gpsimd, nc.scalar]
            gi = 0
            for dgi in range(2):
                for k in range(N):
                    L = S["Lg"].get((dgi, k), 0)
                    if L == 0:
                        continue
                    m, dstoff = S["bins"][dgi][k]
                    sh = SHIFTS[k]
                    base = pstart[S["widx"][(dgi, k)]] + S["rel"][(dgi, k)]
                    src = afl_d[:, :].copy()
                    src.ap = mybir.VecI64Pair(
                        [[DIM0, L], [sh * DIM0, NTC], [1, DIM0]])
                    src.offset = base * DIM0
                    dst = pk[dgi][dstoff:dstoff + L, :, m, :]
                    gd = geng[gi % len(geng)].dma_start(dst, src)
                    gi += 1
                    lo = base + min(0, (NTC - 1) * sh)
                    hi = base + max(0, (NTC - 1) * sh) + L
                    for (p0, p1, dd) in dump_dmas:
                        if p0 * 128 < hi and lo < p1 * 128:
                            add_dep_helper(gd.ins, dd.ins,
                                           reason="aflat roundtrip order")

            # ---------------- stages B + C per chunk ----------------
            binbase = [0, TM[0] * NTC]
            for dgi in range(2):
                d0, dg = D0S[dgi], DGS[dgi]
                tm = TM[dgi]
                for tci in range(NTC):
                    tcw = min(TC, TW - tci * TC)
                    cols = dg * tcw
                    bb = binbase[dgi] + tci * tm
                    wt = wst.tile([128, tm, CMAX], F16, tag=f"ws{dgi}",
                                  name=f"ws{dgi}_{tci}")
                    nc.sync.dma_start(
                        wt[:], wsmp_d[bb:bb + tm].transpose((1, 0, 2)))
                    if dgi == 0 and tci < 4:
                        # keep the PE HAM window warm across the gather gap
                        wm = psA.tile([1, 4], F32, tag="a",
                                      name=f"warmb{tci}")
                        nc.tensor.matmul(wm[:], wt[:, 0, 0:1],
                                         wt[:, 0, 0:4],
                                         start=True, stop=True)
                    x2c = [None] * 4
                    for g in range(2):
                        accs = []
                        for oo in range(2):
                            o = 2 * g + oo
                            acc = psB.tile([128, CMAX], F32, tag=f"b{oo}",
                                           name=f"psb{dgi}_{tci}_{o}")
                            accs.append(acc)
                            for m in range(tm):
                                nc.tensor.matmul(
                                    acc[:, 0:cols],
                                    pk[dgi][:, tci, m,
                                            o * 128:(o + 1) * 128],
                                    wt[:, m, 0:cols],
                                    start=(m == 0), stop=(m == tm - 1),
                                )
                        for oo in range(2):
                            o = 2 * g + oo
                            yt = x2p.tile([128, CMAX], F16, tag=f"x2{o}",
                                          name=f"x2_{dgi}_{tci}_{o}")
                            if oo == 0 or not zb:
                                nc.scalar.activation(
                                    yt[:, 0:cols], accs[oo][:, 0:cols],
                                    lrelu, bias=b0t[:, o:o + 1], scale=1.0)
                            else:
                                # DVE path (bias==0): copy-convert + leaky
                                nc.vector.tensor_copy(yt[:, 0:cols],
                                                      accs[oo][:, 0:cols])
                                nc.vector.scalar_tensor_tensor(
                                    yt[:, 0:cols], yt[:, 0:cols], 0.01,
                                    yt[:, 0:cols], mybir.AluOpType.mult,
                                    mybir.AluOpType.max)
                            x2c[o] = yt
                            if debug and dgi == 0 and tci == 2:
                                nc.sync.dma_start(
                                    dbg_x2[o, :, 0:cols],
                                    x2c[o][:, 0:cols])
                    psc = psC.tile([128, CMAX], F32, tag="c",
                                   name=f"psc{dgi}_{tci}")
                    for o in range(4):
                        nc.tensor.matmul(
                            psc[:, 0:cols],
                            w1t[:, o * DIM1:(o + 1) * DIM1],
                            x2c[o][:, 0:cols],
                            start=(o == 0), stop=(o == 3))
                    nc.scalar.activation(
                        pad[:, 1 + d0:1 + d0 + dg,
                            1 + tci * TC:1 + tci * TC + tcw],
                        psc[:, 0:cols].rearrange("p (a b) -> p a b", a=dg),
                        lrelu, bias=b1t[:], scale=1.0)

            # ---------------- stage D (3x3 conv) + E ----------------
            if debug:
                nc.sync.dma_start(
                    dbg_pad[:], pad[:].rearrange("p a b -> p (a b)"))
            out_sb = outp.tile([1, D * TW], F32, tag="os", name="out_sb")
            x4cs = [None] * NDCH

            def stage_e(dc):
                d0 = dc * DCH
                fw = min(DCH, D - d0) * TW
                pse = psC.tile([1, DCH * TW], F32, tag="c", name=f"pse{dc}")
                nc.tensor.matmul(pse[:, 0:fw], w3t[:], x4cs[dc][:, 0:fw],
                                 start=True, stop=True)
                nc.scalar.activation(
                    out_sb[:, d0 * TW:d0 * TW + fw], pse[:, 0:fw],
                    sigm, bias=b3t[:], scale=1.0)

            # all conv chunks first (Lrelu era), then all sigmoids: avoids
            # ACT function-table thrash from Lrelu/Sigmoid interleaving
            for dc in range(NDCH):
                d0 = dc * DCH
                nd = min(DCH, D - d0)
                fw = nd * TW
                psd = psA.tile([128, DCH * TW], F32, tag="a", name=f"psd{dc}")
                for j in range(9):
                    dy, dx = j // 3, j % 3
                    nc.tensor.matmul(
                        psd[:, 0:fw],
                        w2t[:, j * DIM1:(j + 1) * DIM1],
                        pad[:, d0 + dy:d0 + dy + nd, dx:dx + TW],
                        start=(j == 0), stop=(j == 8),
                    )
                x4c = x4p.tile([128, DCH * TW], F16, tag=f"x4_{dc}",
                               name=f"x4_{dc}")
                nc.scalar.activation(x4c[:, 0:fw], psd[:, 0:fw], lrelu,
                                     bias=b2t[:], scale=1.0)
                x4cs[dc] = x4c
                if debug and dc == 0:
                    nc.sync.dma_start(dbg_x4[:], x4c[:])
            for dc in range(NDCH):
                stage_e(dc)
            nc.scalar.dma_start(out_d[:], out_sb[:])
    _legalize_waits(nc)
    return nc


_PROGRAM = None
_STRUCT = None


def _get_structure(smp):
    global _STRUCT
    if _STRUCT is None:
        _STRUCT = build_structure(smp)
    return _STRUCT


def _get_program(S, zb=True, debug=False):
    global _PROGRAM
    key = (_structure_key(S), zb, debug)
    if _PROGRAM is None or _PROGRAM[0] != key:
        _PROGRAM = (key, _build_program(S, zb=zb, debug=debug))
    return _PROGRAM[1]


def _host_data(S, feature, smp, w0, b0, w1, b1, w2, b2, w3, b3):
    """Build per-core input maps."""
    wtot = S["wtot"]
    TM = S["TM"]
    TB = (TM[0] + TM[1]) * NTC

    w0p = np.ascontiguousarray(
        np.asarray(w0, np.float32).transpose(1, 2, 0)    # (C, N, DIM0)
        .reshape(2, 128, N * DIM0)).astype(np.float16)
    w1p = np.ascontiguousarray(
        np.asarray(w1, np.float32).T.reshape(4, 128, DIM1)).astype(np.float16)
    w2p = np.ascontiguousarray(
        np.asarray(w2, np.float32).transpose(2, 3, 1, 0).reshape(
            9, DIM1, DIM1)).astype(np.float16)
    w3p = np.ascontiguousarray(
        np.asarray(w3, np.float32).T).astype(np.float16)    # (128, 1)
    b0p = np.ascontiguousarray(
        np.asarray(b0, np.float32).reshape(4, 128, 1))
    b1p = np.asarray(b1, np.float32).reshape(128, 1)
    b2p = np.asarray(b2, np.float32).reshape(128, 1)
    b3p = np.asarray(b3, np.float32).reshape(1, 1)

    feature = np.asarray(feature, np.float32)

    # per-th packed wsmp
    wsmp_th = []
    for th in range(2):
        t0 = th * 128
        ws = np.zeros((TB, 128, CMAX), np.float32)
        binbase = [0, TM[0] * NTC]
        for dgi, (d0, dg) in enumerate(zip(D0S, DGS)):
            for k in range(N):
                L = S["Lu"][(dgi, k)]
                if L == 0:
                    continue
                m, dstoff = S["bins"][dgi][k]
                sh = SHIFTS[k]
                sub = smp[:, k, d0:d0 + dg, :]          # (T, dg, T)
                for tci in range(NTC):
                    tcw = min(TC, TW - tci * TC)
                    lo = S["off"][(S["widx"][(dgi, k)], th)] + \
                        S["rel"][(dgi, k)] + tci * sh
                    taus = lo + np.arange(L)
                    tmask = (taus >= 0) & (taus < T)
                    tcl = np.clip(taus, 0, T - 1)
                    tpos = t0 - 1 + tci * TC + np.arange(tcw)
                    pmask = (tpos >= 0) & (tpos < T)
                    tpl = np.clip(tpos, 0, T - 1)
                    blk = sub[tcl][:, :, tpl]            # (L, dg, tcw)
                    blk = blk * tmask[:, None, None] * pmask[None, None, :]
                    ws[binbase[dgi] + tci * TM[dgi] + m,
                       dstoff:dstoff + L, 0:dg * tcw] = blk.reshape(L, -1)
        wsmp_th.append(ws.astype(np.float16))

    in_maps = []
    for core in range(8):
        b, th = core // 2, core % 2
        fs = np.zeros((C_IN, wtot), np.float32)
        for wi in range(len(S["wids"])):
            ofk = S["off"][(wi, th)]
            u0 = max(0, -ofk)
            u1 = min(S["W"][wi], T - ofk)
            if u1 > u0:
                fs[:, S["wstart"][wi] + u0:S["wstart"][wi] + u1] = \
                    feature[b][:, ofk + u0:ofk + u1]
        in_maps.append({
            "fs": np.ascontiguousarray(
                fs.reshape(2, 128, wtot)).astype(np.float16),
            "w0p": w0p,
            "wsmp": wsmp_th[th],
            "w1p": w1p,
            "w2p": w2p,
            "w3p": w3p,
            "b0": b0p,
            "b1": b1p,
            "b2": b2p,
            "b3": b3p,
        })
    return in_maps


def kernel(feature, smp_weight, w0, b0, w1, b1, w2, b2, w3, b3,
           _trace=False, _debug=False):
    smp = np.asarray(smp_weight, np.float32).reshape(T, N, D, T)
    S = _get_structure(smp)
    zb = all(float(np.abs(np.asarray(x)).max()) == 0.0 for x in (b0, b1))
    nc = _get_program(S, zb=zb, debug=_debug)
    in_maps = _host_data(S, feature, smp, w0, b0, w1, b1, w2, b2, w3, b3)
    res = run_bass_kernel_spmd(nc, in_maps, core_ids=list(range(8)),
                               trace=_trace)
    out = np.empty((B, D, T), dtype=np.float32)
    for core in range(8):
        b, th = core // 2, core % 2
        full = res.results[core]["out"].reshape(D, TW)
        out[b, :, th * 128:(th + 1) * 128] = full[:, 1:TW - 1]
    if _trace or _debug:
        return out, res
    return out


# revision 4
# speedup vs baseline: 1.2318x; 1.1260x over previous
"""Trainium2 Bass kernel for nn_BoundaryModule_38422777430159 (v2).

Reference (B=4, C=256, T=256, N=10, D=40, DIM0=512, DIM1=128):
  x1 = sample(feature)            # (B,C,N,D,T) via (T, N*D*T) smp matmul
  x2 = leaky(einsum('bcndt,ocn->bodt', x1, w0) + b0)
  x3 = leaky(w1 @ x2 + b1)
  x4 = leaky(conv3x3(x3, w2) + b2)
  out = sigmoid(w3 @ x4 + b3)     # (B, D, T)

v2 strategy (8 cores SPMD, core = (b, t-half), TW=130 incl halo):
  The sampling matrix columns (k,d,t) have tau-support that is an interval
  moving AFFINELY in t with integer stride shift_k = 27-4k per Tc=18 t-chunk.
  Per core:
    A'[k]  = windowed featT @ w0[k]  (PE, f16 in / fp32 psum), windows chosen
             per-core (th) so the program structure is th-independent.
    A' -> DRAM (flat rows) -> strided overlapping-window gather DMAs pack the
             needed (k,tau) rows densely into per-(d-group) stationary tiles
             [128, NTC, TM, 512] (DRAM is flat, so arbitrary row packing is
             free; compute engines cannot shift partitions by non-32).
    x2     = packed_A.T @ wsmp_packed   (TM=2 matmuls per (chunk, o-chunk))
    x3     = w1.T @ x2 (per chunk) -> written straight into the conv pad buf
    x4     = 3x3 conv (9 taps), out = sigmoid(w3.T x4 + b3).
  All 16-bit operands are fp16; PSUM accumulates fp32. Activations+bias+
  evictions are fused single ACT-engine ops (Lrelu/Sigmoid).
"""
import os
import sys

for _p in ("/opt/trn_rl_repo", "/root/.axon_site/_ro/trn_rl_repo"):
    if os.path.isdir(_p) and _p not in sys.path:
        sys.path.append(_p)

import numpy as np

import concourse.bass as bass
import concourse.tile as tile
from concourse import mybir
from concourse.bass_utils import run_bass_kernel_spmd
from concourse.tile_rust import add_dep_helper

T = 256
N = 10
D = 40
B = 4
C_IN = 256
DIM0 = 512
DIM1 = 128
TW = 130

TC = 18
NTC = (TW + TC - 1) // TC          # 8
DGS = (21, 19)
D0S = (0, 21)
CMAX = DGS[0] * TC                 # 378
SHIFTS = [27 - 4 * k for k in range(N)]
DCH = 3
NDCH = (D + DCH - 1) // DCH        # 14

F32 = mybir.dt.float32
F16 = mybir.dt.float16
LRELU = None  # resolved at build: mybir.ActivationFunctionType.Lrelu


def _legalize_waits(nc, limit=1):
    """Walrus build allows one embedded sync wait per real instruction;
    move the excess onto standalone NoOp wait-carriers."""
    for f in nc.m.functions:
        for bb in f.blocks:
            out = []
            changed = False
            for inst in bb.instructions:
                si = inst.sync_info
                ty = type(inst).__name__
                if (si and si.on_wait and len(si.on_wait) > limit
                        and ty not in ("InstEventSemaphore", "InstNoOp")):
                    keep = si.on_wait[-limit:]
                    for w in si.on_wait[:-limit]:
                        out.append(mybir.InstNoOp(
                            name=f"waitnop-{nc.next_id()}",
                            sync_info=mybir.SyncInfo(on_wait=[w], on_update=[]),
                            bass_nofuse=True,
                            engine=inst.engine,
                        ))
                    inst.sync_info = mybir.SyncInfo(
                        on_wait=keep, on_update=si.on_update)
                    changed = True
                out.append(inst)
            if changed:
                bb.instructions = out


def _try_two_bins(lens, cap=128):
    items = [(L, i) for i, L in enumerate(lens) if L > 0]
    R = sum(L for L, _ in items)
    if R > 2 * cap:
        return None
    # subset-sum: find subset with sum in [R-cap, cap]
    reach = {0: ()}
    for L, i in items:
        new = {}
        for s, sel in reach.items():
            if s + L <= cap and s + L not in reach:
                new[s + L] = sel + (i,)
        reach.update(new)
    best = None
    for s, sel in reach.items():
        if R - s <= cap and (best is None or s > best[0]):
            best = (s, sel)
    if best is None:
        return None
    sel = set(best[1])
    assign = {}
    off0 = off1 = 0
    for L, i in items:
        if i in sel:
            assign[i] = (0, off0)
            off0 += L
        else:
            assign[i] = (1, off1)
            off1 += L
    return 2, assign


def _ffd_bins(lens, cap=128):
    """Pack items into bins of `cap`. Tries an exact 2-bin split (padding an
    item by up to +3 rows to fix subset-sum parity), else FFD."""
    for extra_i in range(-1, len(lens)):
        for delta in ([0] if extra_i < 0 else [1, 2, 3]):
            ll = list(lens)
            if extra_i >= 0:
                if ll[extra_i] == 0:
                    continue
                ll[extra_i] += delta
            r = _try_two_bins(ll, cap)
            if r is not None:
                return r[0], r[1], ll
    order = sorted(range(len(lens)), key=lambda i: -lens[i])
    bins = []
    assign = {}
    for i in order:
        L = lens[i]
        if L == 0:
            continue
        placed = False
        for bi in range(len(bins)):
            if bins[bi] + L <= cap:
                assign[i] = (bi, bins[bi])
                bins[bi] += L
                placed = True
                break
        if not placed:
            bins.append(L)
            assign[i] = (len(bins) - 1, 0)
    return len(bins), assign, list(lens)


def build_structure(smp):
    """smp: (T, N, D, T) float32.  Returns the (th-independent) program
    structure plus per-th offsets for host data construction."""
    # per (dgi, k, th): linearized span (alo at tci=0, len) from actual data
    alo = {}
    ln = {}
    for dgi, (d0, dg) in enumerate(zip(D0S, DGS)):
        for k in range(N):
            sh = SHIFTS[k]
            for th in range(2):
                t0 = th * 128
                spans = []
                for tci in range(NTC):
                    ts = t0 - 1 + tci * TC
                    te = min(t0 - 1 + min(TW, (tci + 1) * TC), T)
                    tsc = max(ts, 0)
                    sub = smp[:, k, d0:d0 + dg, tsc:te]
                    nz = np.nonzero(np.abs(sub).max(axis=(1, 2)) > 0)[0]
                    spans.append((int(nz.min()), int(nz.max()) + 1)
                                 if len(nz) else None)
                valid = [(tci, s) for tci, s in enumerate(spans) if s]
                if not valid:
                    alo[(dgi, k, th)] = 0
                    ln[(dgi, k, th)] = 0
                    continue
                a = min(s[0] - tci * sh for tci, s in valid)
                L = max(s[1] - (a + tci * sh) for tci, s in valid)
                alo[(dgi, k, th)] = int(a)
                ln[(dgi, k, th)] = int(L)

    # Window selection: a window is shared across d-groups when the
    # th-unified union costs little; otherwise split per (k, dgi) (clipping
    # asymmetry between t-halves can inflate the shared union a lot).
    NDG = len(DGS)
    wids = []            # list of (k, dgi-or-None)
    widx = {}            # (dgi, k) -> window index
    off = {}             # (wi, th) -> absolute tau of window start
    rel = {}
    Lu = {}
    W = []
    for k in range(N):
        sh = SHIFTS[k]
        sh_lo = min(0, (NTC - 1) * sh)
        # shared-window candidate
        offs = {th: min(alo[(dgi, k, th)] + sh_lo for dgi in range(NDG))
                for th in range(2)}
        sh_rel = {}
        sh_Lu = {}
        for dgi in range(NDG):
            r = min(alo[(dgi, k, th)] - offs[th] for th in range(2))
            L = max(alo[(dgi, k, th)] - offs[th] + ln[(dgi, k, th)]
                    for th in range(2)) - r
            sh_rel[dgi] = int(r)
            sh_Lu[dgi] = int(L)
        split_Lu = {dgi: max(ln[(dgi, k, th)] for th in range(2))
                    for dgi in range(NDG)}
        if sum(sh_Lu.values()) - sum(split_Lu.values()) > 6:
            for dgi in range(NDG):
                wi = len(wids)
                wids.append((k, dgi))
                widx[(dgi, k)] = wi
                for th in range(2):
                    off[(wi, th)] = int(alo[(dgi, k, th)] + sh_lo)
                rel[(dgi, k)] = int(-sh_lo)
                Lu[(dgi, k)] = int(split_Lu[dgi])
                W.append(int(rel[(dgi, k)] + max(0, (NTC - 1) * sh)
                             + Lu[(dgi, k)]))
        else:
            wi = len(wids)
            wids.append((k, None))
            for dgi in range(NDG):
                widx[(dgi, k)] = wi
                rel[(dgi, k)] = sh_rel[dgi]
                Lu[(dgi, k)] = sh_Lu[dgi]
            for th in range(2):
                off[(wi, th)] = int(offs[th])
            W.append(int(max(rel[(dgi, k)] + max(0, (NTC - 1) * sh)
                             + Lu[(dgi, k)] for dgi in range(NDG))))
            wmin = min(rel[(dgi, k)] + sh_lo for dgi in range(NDG))
            assert wmin >= 0, (k, wmin)

    # bins per dgi; gather lengths (Lg) are filler-extended so each bin is
    # exactly 128 rows -> gather DMAs fully cover the packed tiles (no memset)
    TM = []
    binassign = []
    Lg = {}
    for dgi in range(len(DGS)):
        lens = [Lu[(dgi, k)] for k in range(N)]
        nb, assign, plens = _ffd_bins(lens)
        TM.append(nb)
        binassign.append(assign)
        for k in range(N):
            if plens[k] > 0:
                Lg[(dgi, k)] = int(plens[k])
                need = rel[(dgi, k)] + max(0, (NTC - 1) * SHIFTS[k]) + \
                    int(plens[k])
                wi = widx[(dgi, k)]
                W[wi] = max(W[wi], need)

    wstart = [int(x) for x in np.concatenate([[0], np.cumsum(W)])]
    wtot = int(wstart[-1])
    W = [int(x) for x in W]

    # A' pieces (per window, <=128 rows each)
    pieces = []
    for wi, (k, _dg) in enumerate(wids):
        o = 0
        while o < W[wi]:
            plen = min(128, W[wi] - o)
            pieces.append((k, wi, int(wstart[wi] + o), o, plen))
            o += plen

    # verify coverage: every nonzero of every th is inside its segment
    for dgi, (d0, dg) in enumerate(zip(D0S, DGS)):
        for k in range(N):
            sh = SHIFTS[k]
            for th in range(2):
                t0 = th * 128
                for tci in range(NTC):
                    ts = t0 - 1 + tci * TC
                    te = min(t0 - 1 + min(TW, (tci + 1) * TC), T)
                    tsc = max(ts, 0)
                    sub = smp[:, k, d0:d0 + dg, tsc:te]
                    nz = np.nonzero(np.abs(sub).max(axis=(1, 2)) > 0)[0]
                    if not len(nz):
                        continue
                    lo = off[(widx[(dgi, k)], th)] + rel[(dgi, k)] + tci * sh
                    assert lo <= nz.min() and nz.max() < lo + Lu[(dgi, k)], \
                        (dgi, k, th, tci, lo, nz.min(), nz.max())

    return dict(alo=alo, ln=ln, off=off, W=W, rel=rel, Lu=Lu, Lg=Lg,
                wstart=wstart, wtot=wtot, pieces=pieces, TM=TM,
                bins=binassign, wids=wids, widx=widx)


def _structure_key(S):
    return (tuple(S["W"]), tuple(S["TM"]), tuple(S["wids"]),
            tuple(sorted((k, v) for k, v in S["rel"].items())),
            tuple(sorted((k, v) for k, v in S["Lu"].items())),
            tuple(sorted((k, v) for k, v in S["Lg"].items())),
            tuple(S["pieces"]),
            tuple(sorted((k, (v[0], v[1])) for k, v in S["bins"][0].items())),
            tuple(sorted((k, (v[0], v[1])) for k, v in S["bins"][1].items())))


def _build_program(S, zb=True, debug=False):
    lrelu = mybir.ActivationFunctionType.Lrelu
    sigm = mybir.ActivationFunctionType.Sigmoid
    copyf = mybir.ActivationFunctionType.Copy

    wtot = S["wtot"]
    TM = S["TM"]
    TB = (TM[0] + TM[1]) * NTC      # total moving bins
    nc = bass.Bass(trn_type="TRN2")

    fs_d = nc.dram_tensor("fs", [2, 128, wtot], F16, kind="ExternalInput")
    w0_d = nc.dram_tensor("w0p", [2, 128, N * DIM0], F16, kind="ExternalInput")
    wsmp_d = nc.dram_tensor("wsmp", [TB, 128, CMAX], F16, kind="ExternalInput")
    w1_d = nc.dram_tensor("w1p", [4, 128, DIM1], F16, kind="ExternalInput")
    w2_d = nc.dram_tensor("w2p", [9, 128, DIM1], F16, kind="ExternalInput")
    w3_d = nc.dram_tensor("w3p", [128, 1], F16, kind="ExternalInput")
    b0_d = nc.dram_tensor("b0", [4, 128, 1], F32, kind="ExternalInput")
    b1_d = nc.dram_tensor("b1", [128, 1], F32, kind="ExternalInput")
    b2_d = nc.dram_tensor("b2", [128, 1], F32, kind="ExternalInput")
    b3_d = nc.dram_tensor("b3", [1, 1], F32, kind="ExternalInput")
    afl_d = nc.dram_tensor("aflat", [len(S["pieces"]) * 128, DIM0], F16,
                           kind="Internal")
    out_d = nc.dram_tensor("out", [1, D * TW], F32, kind="ExternalOutput")
    if debug:
        dbg_x2 = nc.dram_tensor("dbg_x2", [4, 128, CMAX], F16,
                                kind="ExternalOutput")
        dbg_pad = nc.dram_tensor("dbg_pad", [128, (D + 2) * (TW + 2)], F16,
                                 kind="ExternalOutput")
        dbg_x4 = nc.dram_tensor("dbg_x4", [128, DCH * TW], F16,
                                kind="ExternalOutput")

    with tile.TileContext(nc) as tc:
        with (
            tc.tile_pool(name="inp", bufs=1) as inp,
            tc.tile_pool(name="apool", bufs=3) as apool,
            tc.tile_pool(name="pk", bufs=1) as pkp,
            tc.tile_pool(name="wst", bufs=3) as wst,
            tc.tile_pool(name="x2p", bufs=2) as x2p,
            tc.tile_pool(name="x3p", bufs=1) as x3p,
            tc.tile_pool(name="x4p", bufs=1) as x4p,
            tc.tile_pool(name="outp", bufs=1) as outp,
            tc.tile_pool(name="psA", bufs=2, space="PSUM") as psA,
            tc.tile_pool(name="psB", bufs=2, space="PSUM") as psB,
            tc.tile_pool(name="psC", bufs=2, space="PSUM") as psC,
        ):
            # ---------------- input DMAs ----------------
            fs_sb = []
            fs_dmas = []
            for c in range(2):
                t_ = inp.tile([128, wtot], F16, tag=f"fs{c}", name=f"fs{c}")
                fs_dmas.append(nc.sync.dma_start(t_[:], fs_d[c]))
                fs_sb.append(t_)
            w0_sb = []
            w0_dmas = {}
            for c in range(2):
                t_ = inp.tile([128, N * DIM0], F16, tag=f"w0{c}",
                              name=f"w0{c}")
                w0_sb.append(t_)
                eng = nc.sync if c == 0 else nc.scalar
                for k in range(N):
                    w0_dmas[(c, k)] = eng.dma_start(
                        t_[:, k * DIM0:(k + 1) * DIM0],
                        w0_d[c, :, k * DIM0:(k + 1) * DIM0])
            w1t = inp.tile([128, 4 * DIM1], F16, tag="w1", name="w1_sb")
            nc.sync.dma_start(w1t[:].rearrange("p (a b) -> p a b", a=4),
                              w1_d[:].transpose((1, 0, 2)))
            w2t = inp.tile([128, 9 * DIM1], F16, tag="w2", name="w2_sb")
            nc.sync.dma_start(w2t[:].rearrange("p (a b) -> p a b", a=9),
                              w2_d[:].transpose((1, 0, 2)))
            w3t = inp.tile([128, 1], F16, tag="w3", name="w3_sb")
            nc.sync.dma_start(w3t[:], w3_d[:])
            b0t = inp.tile([128, 4], F32, tag="b0", name="b0_sb")
            nc.sync.dma_start(b0t[:].rearrange("p (a b) -> p a b", b=1),
                              b0_d[:].transpose((1, 0, 2)))
            b1t = inp.tile([128, 1], F32, tag="b1", name="b1_sb")
            nc.sync.dma_start(b1t[:], b1_d[:])
            b2t = inp.tile([128, 1], F32, tag="b2", name="b2_sb")
            nc.sync.dma_start(b2t[:], b2_d[:])
            b3t = inp.tile([1, 1], F32, tag="b3", name="b3_sb")
            nc.sync.dma_start(b3t[:], b3_d[:])

            # packed stationary tiles (one per d-group), zeroed once (f32
            # view: 2x faster) so bin gap rows can never be NaN
            pk = []
            for dgi in range(2):
                t_ = pkp.tile([128, NTC, TM[dgi], DIM0], F16,
                              tag=f"pk{dgi}", name=f"pk{dgi}")
                nc.vector.memset(
                    t_[:].rearrange("p a b f -> p (a b f)").bitcast(F32), 0.0)
                pk.append(t_)
            # conv pad buffer, zeroed once (f32 view: 2x faster memset)
            pad = x3p.tile([128, D + 2, TW + 2], F16, tag="pad", name="pad")
            nc.vector.memset(
                pad[:].rearrange("p a b -> p (a b)").bitcast(F32), 0.0)

            # warm-up accumulation group riding on input DMAs (keeps the PE
            # HAM window busy through the DMA prologue)
            warm = psC.tile([1, 4], F32, tag="c", name="warm_ps")
            nc.tensor.matmul(warm[:], fs_sb[0][:, 0:1], fs_sb[0][:, 0:4],
                             start=True, stop=False)
            nc.tensor.matmul(warm[:], fs_sb[1][:, 0:1], fs_sb[1][:, 0:4],
                             start=False, stop=False)
            for c in range(2):
                for k in (0, 3, 6, 9):
                    nc.tensor.matmul(
                        warm[:], w0_sb[c][:, k * DIM0:k * DIM0 + 1],
                        w0_sb[c][:, k * DIM0:k * DIM0 + 4],
                        start=False, stop=(c == 1 and k == 9))

            # ---------------- stage A -> one big SBUF tile + 2 dump DMAs --
            npc = len(S["pieces"])
            aev = apool.tile([128, npc * DIM0], F16, tag="aev", name="aev")
            phalf = (npc + 1) // 2
            for pi, (k, wi, absrow, relrow, plen) in enumerate(S["pieces"]):
                ps = psA.tile([128, DIM0], F32, tag="a", name=f"psa{pi}")
                for c in range(2):
                    nc.tensor.matmul(
                        ps[0:plen, :],
                        fs_sb[c][:, S["wstart"][wi] + relrow:
                                 S["wstart"][wi] + relrow + plen],
                        w0_sb[c][:, k * DIM0:(k + 1) * DIM0],
                        start=(c == 0), stop=(c == 1),
                    )
                nc.vector.tensor_copy(
                    aev[0:plen, pi * DIM0:(pi + 1) * DIM0], ps[0:plen, :])
            # dump halves: pieces are 128-row aligned in afl (piece pi ->
            # afl rows [pstart(pi)*... ]) -- afl layout is piece-major
            dump_dmas = []
            qs = sorted({0, npc // 4, npc // 2, (3 * npc) // 4, npc})
            for h, (p0, p1) in enumerate(zip(qs, qs[1:])):
                src = aev[:, p0 * DIM0:p1 * DIM0].rearrange(
                    "p (a f) -> p a f", f=DIM0)
                dst = afl_d[p0 * 128:p1 * 128, :].rearrange(
                    "(a p) f -> p a f", p=128)
                eng = nc.sync if h % 2 == 0 else nc.scalar
                dump_dmas.append((p0, p1, eng.dma_start(dst, src)))

            # ---------------- gathers: aflat -> packed stationaries -------
            # afl rows are PIECE-padded: window k starts at piece boundary
            pstart = {}
            for pi, (k, wi, absrow, relrow, plen) in enumerate(S["pieces"]):
                if relrow == 0:
                    pstart[wi] = pi * 128
            # tci-half granularity, ordered (dg0-h0, dg0-h1, dg1-h0, dg1-h1):
            # stage B's first chunks start while later halves still drain
            geng = [nc.gpsimd, nc.scalar]
            gi = 0
            HH = NTC // 2
            for dgi in range(2):
                for hh in range(2):
                    for k in range(N):
                        L = S["Lg"].get((dgi, k), 0)
                        if L == 0:
                            continue
                        m, dstoff = S["bins"][dgi][k]
                        sh = SHIFTS[k]
                        base = (pstart[S["widx"][(dgi, k)]]
                                + S["rel"][(dgi, k)] + hh * HH * sh)
                        src = afl_d[:, :].copy()
                        src.ap = mybir.VecI64Pair(
                            [[DIM0, L], [sh * DIM0, HH], [1, DIM0]])
                        src.offset = base * DIM0
                        dst = pk[dgi][dstoff:dstoff + L,
                                      hh * HH:(hh + 1) * HH, m, :]
                        gd = geng[gi % len(geng)].dma_start(dst, src)
                        gi += 1
                        lo = base + min(0, (HH - 1) * sh)
                        hi = base + max(0, (HH - 1) * sh) + L
                        for (p0, p1, dd) in dump_dmas:
                            if p0 * 128 < hi and lo < p1 * 128:
                                add_dep_helper(gd.ins, dd.ins,
                                               reason="aflat roundtrip order")

            # ---------------- stages B + C per chunk ----------------
            binbase = [0, TM[0] * NTC]
            for dgi in range(2):
                d0, dg = D0S[dgi], DGS[dgi]
                tm = TM[dgi]
                for tci in range(NTC):
                    tcw = min(TC, TW - tci * TC)
                    cols = dg * tcw
                    bb = binbase[dgi] + tci * tm
                    wt = wst.tile([128, tm, CMAX], F16, tag=f"ws{dgi}",
                                  name=f"ws{dgi}_{tci}")
                    nc.sync.dma_start(
                        wt[:], wsmp_d[bb:bb + tm].transpose((1, 0, 2)))
                    if dgi == 0 and tci < 4:
                        # keep the PE HAM window warm across the gather gap
                        wm = psA.tile([1, 4], F32, tag="a",
                                      name=f"warmb{tci}")
                        nc.tensor.matmul(wm[:], wt[:, 0, 0:1],
                                         wt[:, 0, 0:4],
                                         start=True, stop=True)
                    x2c = [None] * 4
                    for g in range(2):
                        accs = []
                        for oo in range(2):
                            o = 2 * g + oo
                            acc = psB.tile([128, CMAX], F32, tag=f"b{oo}",
                                           name=f"psb{dgi}_{tci}_{o}")
                            accs.append(acc)
                            for m in range(tm):
                                nc.tensor.matmul(
                                    acc[:, 0:cols],
                                    pk[dgi][:, tci, m,
                                            o * 128:(o + 1) * 128],
                                    wt[:, m, 0:cols],
                                    start=(m == 0), stop=(m == tm - 1),
                                )
                        for oo in range(2):
                            o = 2 * g + oo
                            yt = x2p.tile([128, CMAX], F16, tag=f"x2{o}",
                                          name=f"x2_{dgi}_{tci}_{o}")
                            if oo == 0 or not zb:
                                nc.scalar.activation(
                                    yt[:, 0:cols], accs[oo][:, 0:cols],
                                    lrelu, bias=b0t[:, o:o + 1], scale=1.0)
                            else:
                                # DVE path (bias==0): copy-convert + leaky
                                nc.vector.tensor_copy(yt[:, 0:cols],
                                                      accs[oo][:, 0:cols])
                                nc.vector.scalar_tensor_tensor(
                                    yt[:, 0:cols], yt[:, 0:cols], 0.01,
                                    yt[:, 0:cols], mybir.AluOpType.mult,
                                    mybir.AluOpType.max)
                            x2c[o] = yt
                            if debug and dgi == 0 and tci == 2:
                                nc.sync.dma_start(
                                    dbg_x2[o, :, 0:cols],
                                    x2c[o][:, 0:cols])
                    psc = psC.tile([128, CMAX], F32, tag="c",
                                   name=f"psc{dgi}_{tci}")
                    for o in range(4):
                        nc.tensor.matmul(
                            psc[:, 0:cols],
                            w1t[:, o * DIM1:(o + 1) * DIM1],
                            x2c[o][:, 0:cols],
                            start=(o == 0), stop=(o == 3))
                    nc.scalar.activation(
                        pad[:, 1 + d0:1 + d0 + dg,
                            1 + tci * TC:1 + tci * TC + tcw],
                        psc[:, 0:cols].rearrange("p (a b) -> p a b", a=dg),
                        lrelu, bias=b1t[:], scale=1.0)

            # ---------------- stage D (3x3 conv) + E ----------------
            if debug:
                nc.sync.dma_start(
                    dbg_pad[:], pad[:].rearrange("p a b -> p (a b)"))
            out_sb = outp.tile([1, D * TW], F32, tag="os", name="out_sb")
            x4cs = [None] * NDCH

            def stage_e(dc):
                d0 = dc * DCH
                fw = min(DCH, D - d0) * TW
                pse = psC.tile([1, DCH * TW], F32, tag="c", name=f"pse{dc}")
                nc.tensor.matmul(pse[:, 0:fw], w3t[:], x4cs[dc][:, 0:fw],
                                 start=True, stop=True)
                nc.scalar.activation(
                    out_sb[:, d0 * TW:d0 * TW + fw], pse[:, 0:fw],
                    sigm, bias=b3t[:], scale=1.0)

            # all conv chunks first (Lrelu era), then all sigmoids: avoids
            # ACT function-table thrash from Lrelu/Sigmoid interleaving
            for dc in range(NDCH):
                d0 = dc * DCH
                nd = min(DCH, D - d0)
                fw = nd * TW
                psd = psA.tile([128, DCH * TW], F32, tag="a", name=f"psd{dc}")
                for j in range(9):
                    dy, dx = j // 3, j % 3
                    nc.tensor.matmul(
                        psd[:, 0:fw],
                        w2t[:, j * DIM1:(j + 1) * DIM1],
                        pad[:, d0 + dy:d0 + dy + nd, dx:dx + TW],
                        start=(j == 0), stop=(j == 8),
                    )
                x4c = x4p.tile([128, DCH * TW], F16, tag=f"x4_{dc}",
                               name=f"x4_{dc}")
                nc.scalar.activation(x4c[:, 0:fw], psd[:, 0:fw], lrelu,
                                     bias=b2t[:], scale=1.0)
                x4cs[dc] = x4c
                if debug and dc == 0:
                    nc.sync.dma_start(dbg_x4[:], x4c[:])
            for dc in range(NDCH):
                stage_e(dc)
            nc.scalar.dma_start(out_d[:], out_sb[:])
    _legalize_waits(nc)
    return nc


_PROGRAM = None
_STRUCT = None


def _get_structure(smp):
    global _STRUCT
    if _STRUCT is None:
        _STRUCT = build_structure(smp)
    return _STRUCT


def _get_program(S, zb=True, debug=False):
    global _PROGRAM
    key = (_structure_key(S), zb, debug)
    if _PROGRAM is None or _PROGRAM[0] != key:
        _PROGRAM = (key, _build_program(S, zb=zb, debug=debug))
    return _PROGRAM[1]


def _host_data(S, feature, smp, w0, b0, w1, b1, w2, b2, w3, b3):
    """Build per-core input maps."""
    wtot = S["wtot"]
    TM = S["TM"]
    TB = (TM[0] + TM[1]) * NTC

    w0p = np.ascontiguousarray(
        np.asarray(w0, np.float32).transpose(1, 2, 0)    # (C, N, DIM0)
        .reshape(2, 128, N * DIM0)).astype(np.float16)
    w1p = np.ascontiguousarray(
        np.asarray(w1, np.float32).T.reshape(4, 128, DIM1)).astype(np.float16)
    w2p = np.ascontiguousarray(
        np.asarray(w2, np.float32).transpose(2, 3, 1, 0).reshape(
            9, DIM1, DIM1)).astype(np.float16)
    w3p = np.ascontiguousarray(
        np.asarray(w3, np.float32).T).astype(np.float16)    # (128, 1)
    b0p = np.ascontiguousarray(
        np.asarray(b0, np.float32).reshape(4, 128, 1))
    b1p = np.asarray(b1, np.float32).reshape(128, 1)
    b2p = np.asarray(b2, np.float32).reshape(128, 1)
    b3p = np.asarray(b3, np.float32).reshape(1, 1)

    feature = np.asarray(feature, np.float32)

    # per-th packed wsmp
    wsmp_th = []
    for th in range(2):
        t0 = th * 128
        ws = np.zeros((TB, 128, CMAX), np.float32)
        binbase = [0, TM[0] * NTC]
        for dgi, (d0, dg) in enumerate(zip(D0S, DGS)):
            for k in range(N):
                L = S["Lu"][(dgi, k)]
                if L == 0:
                    continue
                m, dstoff = S["bins"][dgi][k]
                sh = SHIFTS[k]
                sub = smp[:, k, d0:d0 + dg, :]          # (T, dg, T)
                for tci in range(NTC):
                    tcw = min(TC, TW - tci * TC)
                    lo = S["off"][(S["widx"][(dgi, k)], th)] + \
                        S["rel"][(dgi, k)] + tci * sh
                    taus = lo + np.arange(L)
                    tmask = (taus >= 0) & (taus < T)
                    tcl = np.clip(taus, 0, T - 1)
                    tpos = t0 - 1 + tci * TC + np.arange(tcw)
                    pmask = (tpos >= 0) & (tpos < T)
                    tpl = np.clip(tpos, 0, T - 1)
                    blk = sub[tcl][:, :, tpl]            # (L, dg, tcw)
                    blk = blk * tmask[:, None, None] * pmask[None, None, :]
                    ws[binbase[dgi] + tci * TM[dgi] + m,
                       dstoff:dstoff + L, 0:dg * tcw] = blk.reshape(L, -1)
        wsmp_th.append(ws.astype(np.float16))

    in_maps = []
    for core in range(8):
        b, th = core // 2, core % 2
        fs = np.zeros((C_IN, wtot), np.float32)
        for wi in range(len(S["wids"])):
            ofk = S["off"][(wi, th)]
            u0 = max(0, -ofk)
            u1 = min(S["W"][wi], T - ofk)
            if u1 > u0:
                fs[:, S["wstart"][wi] + u0:S["wstart"][wi] + u1] = \
                    feature[b][:, ofk + u0:ofk + u1]
        in_maps.append({
            "fs": np.ascontiguousarray(
                fs.reshape(2, 128, wtot)).astype(np.float16),
            "w0p": w0p,
            "wsmp": wsmp_th[th],
            "w1p": w1p,
            "w2p": w2p,
            "w3p": w3p,
            "b0": b0p,
            "b1": b1p,
            "b2": b2p,
            "b3": b3p,
        })
    return in_maps


def kernel(feature, smp_weight, w0, b0, w1, b1, w2, b2, w3, b3,
           _trace=False, _debug=False):
    smp = np.asarray(smp_weight, np.float32).reshape(T, N, D, T)
    S = _get_structure(smp)
    zb = all(float(np.abs(np.asarray(x)).max()) == 0.0 for x in (b0, b1))
    nc = _get_program(S, zb=zb, debug=_debug)
    in_maps = _host_data(S, feature, smp, w0, b0, w1, b1, w2, b2, w3, b3)
    res = run_bass_kernel_spmd(nc, in_maps, core_ids=list(range(8)),
                               trace=_trace)
    out = np.empty((B, D, T), dtype=np.float32)
    for core in range(8):
        b, th = core // 2, core % 2
        full = res.results[core]["out"].reshape(D, TW)
        out[b, :, th * 128:(th + 1) * 128] = full[:, 1:TW - 1]
    if _trace or _debug:
        return out, res
    return out
